# revision 21
# baseline (speedup 1.0000x reference)
"""Causal multi-head attention block (qkv proj + attention + out proj) on 8
Trainium2 NeuronCores.

Sharding: Megatron-style tensor parallel over heads -- 2 heads per core.
Each core computes its heads' Q/K/V projections (column-sharded w_qkv),
causal attention for those heads, and a row-sharded partial of the output
projection.  The host sums the 8 partial outputs and adds b_o.

Device-side layout notes:
 - X^T [C, B*T] (fp16) feeds every matmul contraction dim on SBUF
   partitions with no on-device transposes.  Q^T/K^T come from the
   weight-stationary projection; V is produced keys-major directly by
   using the X^T tile as the stationary operand (out = X_chunk @ Wv), so
   no PE transpose or PSUM evacuation of V^T is needed.
 - Scores are computed transposed (S^T[k, q] = K^T.T @ Q^T per 128-wide
   k block) with the two heads' matmuls row-packed on the PE (partitions
   0:64 / 64:128).  Softmax exp runs on the scalar engine; the
   denominator is an extra all-ones column appended to V (row 64 of the
   attn@V accumulator).
 - The emission order interleaves the next batch's projections (aq) and
   deferred latency-sensitive work (lq: output projection, softmax
   divide finish) into the attention kb-loop so the PE queue never
   drains.  A reserve of projection units is held back to bridge the
   batch-boundary divide chain; the last batch's rcl2-3 projections are
   deferred into its own attention loop (it has no next batch to fill
   with).
 - Softmax divide: one evacuation copy frees the PSUM banks; the
   denominator reciprocal runs directly on the PSUM ones-row
   (reciprocal_approx_fast, fp32), is partition-broadcast in two halves,
   and the two muls are deferred (lq) so they never head-block the DVE
   queue.  Projection evacuations in prologue/batch-end bursts alternate
   between the scalar and vector engines so PSUM recycling is never
   serialized behind the divide chain.
 - The final query chunk of the last batch pipelines its divide + output
   projection in two 256-query halves to shorten the drain tail.
"""

import numpy as np
import ml_dtypes
from collections import deque
from contextlib import ExitStack

import concourse.bass as bass
import concourse.tile as tile
import concourse.mybir as mybir
from concourse import bacc
from concourse.bass_utils import run_bass_kernel_spmd

B, T, C, H, DH = 4, 2048, 1024, 16, 64
NCORES = 8
HPC = H // NCORES            # heads per core = 2
R = B * T                    # 8192 rows
HD = HPC * DH                # 128 local head dims
KT = C // 128                # 8 contraction tiles over C
RC = 512                     # row chunk in qkv stage
QC = 512                     # query chunk in attention
NQC = T // QC                # 4
NKB = T // 128               # 16 key blocks per batch

F32 = mybir.dt.float32
F16 = mybir.dt.float16
I16 = mybir.dt.int16

# Schraudolph fp16-bitcast exp constants (i16 = s*1024*log2(e) + bias)
_EXP_C_MUL = 1024.0 * 1.4426950408889634
_EXP_C_ADD = 1024.0 * 15.0 - 46.1277

LAST_RESULT = None           # BassKernelResults of the most recent run
_CACHED_NC = None


def _emit(nc, tc, xt, wqkv, bqkv, wo, tri, y, use_bias=False):
    Exp = mybir.ActivationFunctionType.Exp
    with ExitStack() as ctx:
        const = ctx.enter_context(tc.tile_pool(name="const", bufs=1))
        bigp = ctx.enter_context(tc.tile_pool(name="bigp", bufs=2))
        xtp = ctx.enter_context(tc.tile_pool(name="xtp", bufs=3))
        vsbp = ctx.enter_context(tc.tile_pool(name="vsbp", bufs=2))
        ptp = ctx.enter_context(tc.tile_pool(name="ptp", bufs=5))
        osbp = ctx.enter_context(tc.tile_pool(name="osbp", bufs=2))
        ystp = ctx.enter_context(tc.tile_pool(name="ystp", bufs=3))
        smallp = ctx.enter_context(tc.tile_pool(name="smallp", bufs=3))
        psP = ctx.enter_context(tc.tile_pool(name="psP", bufs=2, space="PSUM"))
        psS = ctx.enter_context(tc.tile_pool(name="psS", bufs=2, space="PSUM"))
        psO = ctx.enter_context(tc.tile_pool(name="psO", bufs=1, space="PSUM"))

        # ---- constants (issued on idle queues so the sync queue can
        # start streaming x_t immediately) ----
        w_sb = const.tile([128, 3, KT, HD], F16, name="w_sb")
        nc.scalar.dma_start(out=w_sb[:, 0, :, :], in_=wqkv[0, :, :, :])
        # wo/tri/bias loads are deferred behind the first x chunk on the
        # sync queue: they are not needed until the first oproj / diag
        # block, and issuing them at t0 steals DMA bandwidth from the
        # prologue-critical x stream
        wo_sb = const.tile([128, C], F16, name="wo_sb")
        tri_sb = const.tile([128, 128], F16, name="tri_sb")
        b_sb = const.tile([128, 3], F32, name="b_sb")
        if use_bias:
            for m in range(3):
                nc.gpsimd.dma_start(
                    out=b_sb[:, m : m + 1],
                    in_=bqkv[m : m + 1, :].rearrange("a n -> n a"),
                )
        bvv_sb = None
        if use_bias:
            bvv_sb = const.tile([128, HD], F32, name="bvv_sb")
            nc.sync.dma_start(
                out=bvv_sb[:, :],
                in_=bqkv[2:3, :].broadcast_to([128, HD]),
            )

        state = {}

        def alloc_batch(b):
            st = {
                "qt": bigp.tile([128, T], F16, name="qt", tag="qt"),
                "ktt": bigp.tile([128, T], F16, name="ktt", tag="ktt"),
                "vsb": vsbp.tile([128, NKB, 2, 65], F16, name="vsb", tag="vsb"),
                "osb": osbp.tile([128, T], F16, name="osb", tag="osb"),
                "xt": {},
            }
            return st

        def dma_unit(st, b2, rcl, halves=False, prologue=False):
            def f(evac="vector"):
                x_t = xtp.tile([128, KT, RC], F16, name="x_t", tag="xt")
                rc = b2 * (T // RC) + rcl
                eng = nc.sync
                if halves:
                    # subtile deps let the first matmuls start on the
                    # first quarters while the rest are still in flight
                    for kq in range(4):
                        eng.dma_start(
                            out=x_t[:, 2 * kq : 2 * kq + 2, :],
                            in_=xt[:, rc, 2 * kq : 2 * kq + 2, :],
                        )
                else:
                    eng.dma_start(out=x_t[:, :, :], in_=xt[:, rc, :, :])
                st["xt"][rcl] = x_t

            return f

        def ones_unit(st):
            def f(evac="vector"):
                nc.gpsimd.memset(st["vsb"][:, :, :, 64:65], 1.0)

            return f

        def qk_unit(st, rcl, m):
            def f(evac="vector"):
                x_t = st["xt"][rcl]
                ps = psP.tile([128, RC], F32, name="ps_qk", tag="pp")
                for k in range(KT):
                    nc.tensor.matmul(
                        ps[:, :],
                        lhsT=w_sb[:, m, k, :],
                        rhs=x_t[:, k, :],
                        start=(k == 0),
                        stop=(k == KT - 1),
                    )
                dst = (st["qt"] if m == 0 else st["ktt"])[
                    :, rcl * RC : (rcl + 1) * RC
                ]
                if use_bias:
                    nc.vector.tensor_scalar_add(
                        out=dst, in0=ps[:, :], scalar1=b_sb[:, m : m + 1]
                    )
                elif evac == "scalar":
                    nc.scalar.copy(out=dst, in_=ps[:, :])
                else:
                    nc.vector.tensor_copy(out=dst, in_=ps[:, :])

            return f

        def v_unit(st, rcl, rt):
            def f(evac="vector"):
                x_t = st["xt"][rcl]
                ps = psP.tile([128, RC], F32, name="ps_v", tag="pp")
                for k in range(KT):
                    nc.tensor.matmul(
                        ps[:, 0:128],
                        lhsT=x_t[:, k, rt * 128 : (rt + 1) * 128],
                        rhs=w_sb[:, 2, k, :],
                        start=(k == 0),
                        stop=(k == KT - 1),
                    )
                kb = rcl * 4 + rt
                vsb = st["vsb"]
                src = ps[:, 0:128].rearrange("p (h d) -> p h d", h=2)
                if use_bias:
                    # v bias varies along the free (dim) axis here, so a
                    # pre-replicated [128, HD] tile is added elementwise
                    nc.vector.tensor_add(
                        out=ps[:, 0:128], in0=ps[:, 0:128], in1=bvv_sb[:, :]
                    )
                    nc.vector.tensor_copy(out=vsb[:, kb, :, 0:64], in_=src)
                elif evac == "scalar":
                    nc.scalar.copy(out=vsb[:, kb, :, 0:64], in_=src)
                else:
                    nc.vector.tensor_copy(out=vsb[:, kb, :, 0:64], in_=src)

            return f

        def stage_a_units(st, b2, prologue=False, defer_tail=False):
            """Returns (units, deferred_units).  deferred_units (rcl 2-3
            projections) are only split out for the last batch, which has
            no successor to supply fillers for its attention loop."""
            defer = []
            if prologue:
                # first batch is gated on its own first chunk: don't put
                # prefetches ahead of it in the DMA engines
                us = [dma_unit(st, b2, 0, halves=True, prologue=True),
                      ones_unit(st)]
                for rcl in range(4):
                    us.append(qk_unit(st, rcl, 0))
                    if rcl + 1 < 4:
                        us.append(dma_unit(st, b2, rcl + 1, prologue=True))
                    us.append(qk_unit(st, rcl, 1))
                    for rt in range(4):
                        us.append(v_unit(st, rcl, rt))
                return us, defer
            us = [dma_unit(st, b2, 0), dma_unit(st, b2, 1), ones_unit(st)]
            for rcl in range(4):
                tgt = defer if (defer_tail and rcl >= 2) else us
                tgt.append(qk_unit(st, rcl, 0))
                if rcl + 2 < 4:
                    us.append(dma_unit(st, b2, rcl + 2))
                tgt.append(qk_unit(st, rcl, 1))
                for rt in range(4):
                    tgt.append(v_unit(st, rcl, rt))
            return us, defer

        def oproj_unit(st, b, qc, half, split_q=False):
            """Returns a list of two filler units (one per 128-row block)
            sharing one yst tile; the second unit issues the combined
            DMA.  Finer units spread PE filler work more evenly through
            the exp-paced diagonal regions."""
            osb = st["osb"]
            rb0 = 4 * qc + 2 * half
            yst = ystp.tile([128, 2, 2 * RC], F16, name="yst", tag="yst")

            def piece(i):
                def f(evac="vector"):
                    rb = rb0 + i
                    for j in range(2):
                        ps = psP.tile([128, 512], F32, name="ps_o", tag="pp")
                        nc.tensor.matmul(
                            ps[:, :],
                            lhsT=osb[:, rb * 128 : (rb + 1) * 128],
                            rhs=wo_sb[:, j * 512 : (j + 1) * 512],
                            start=True,
                            stop=True,
                        )
                        dst = yst[:, i, j * 512 : (j + 1) * 512]
                        if split_q and j == 1:
                            # drain tail: ACT is idle once the last exp
                            # is done, so split evacuation across engines
                            nc.scalar.copy(out=dst, in_=ps[:, :])
                        else:
                            nc.vector.tensor_copy(out=dst, in_=ps[:, :])
                    if split_q:
                        nc.sync.dma_start(
                            out=y[b * T + rb * 128 : b * T + (rb + 1) * 128, :],
                            in_=yst[:, i, :],
                        )
                    elif i == 1:
                        nc.sync.dma_start(
                            out=y[
                                b * T + rb0 * 128 : b * T + (rb0 + 2) * 128, :
                            ].rearrange("(i p) c -> p i c", i=2),
                            in_=yst[:, :, :],
                        )

                return f

            return [piece(0), piece(1)]

        # two filler streams woven into the attention kb-loop:
        #  - aq: next batch's projections (independent, always ready)
        #  - lq: latency-sensitive deferred work (softmax-divide finish,
        #    output projection) that must not reach an engine queue before
        #    its upstream chain has had time to complete
        #  - b3q: the last batch's own deferred rcl2-3 projections
        aq = deque()
        lq = deque()  # entries: (kind, fn); 'df' = divide-finish, 'op' = oproj
        b3q = deque()
        cur_b = [0]
        RESERVE = 2   # aq units held back to cushion the batch-end chain

        def pop_filler(slot):
            # the deferred divide-finish must not reach the DVE queue
            # until its reciprocal-broadcast inputs have landed (~2 slots)
            if lq and lq[0][0] == "df" and slot >= 2:
                lq.popleft()[1]()
            elif b3q and cur_b[0] == B - 1 and (slot % 2 == 0 or not lq):
                b3q.popleft()()
            elif lq and lq[0][0] != "df":
                lq.popleft()[1]()
            elif len(aq) > RESERVE:
                aq.popleft()()

        def divide_chain(st, o_ps, qc, lo, hi, defer=True):
            """Softmax divide for query columns [lo:hi) of this qc.
            Mid-batch: evacuate the attn@V accumulator (frees the PSUM
            banks), spread the ones-row across 32 lanes by DMA (sync
            queue -- it is idle), reciprocal there, despread, then
            partition-broadcast in two halves.  The two muls write both
            head halves of osb directly (engines shift partition bases
            on plain tensor ops, so the h1 mul writes partitions 64:128
            straight from base-0 inputs -- no shift DMA)."""
            osb = st["osb"]
            n = hi - lo
            if lo == 0:
                st["onum"] = smallp.tile([65, 2, QC], F32, name="onum", tag="on")
            onum = st["onum"]
            nc.vector.tensor_copy(
                out=onum[:, :, lo:hi], in_=o_ps[:, :, lo:hi]
            )
            sp = smallp.tile([32, QC // 8], F32, name="sp", tag="sp")
            nc.sync.dma_start(out=sp[:, 0 : n // 16],
                              in_=onum[64:65, :, lo:hi])
            sph = smallp.tile([32, QC // 8], F16, name="sph", tag="sph")
            with nc.allow_low_precision(
                reason="softmax reciprocal broadcast in fp16"
            ):
                nc.vector.reciprocal(
                    out=sph[:, 0 : n // 16], in_=sp[:, 0 : n // 16]
                )
            srow = smallp.tile([1, 2, QC], F16, name="srow", tag="srow")
            nc.sync.dma_start(out=srow[0:1, :, lo:hi],
                              in_=sph[:, 0 : n // 16])
            bch = smallp.tile([64, 2, QC], F16, name="bch", tag="bch")
            half = n // 2
            nc.gpsimd.partition_broadcast(
                out_ap=bch[:, :, lo : lo + half],
                in_ap=srow[0:1, :, lo : lo + half],
            )
            nc.gpsimd.partition_broadcast(
                out_ap=bch[:, :, lo + half : hi],
                in_ap=srow[0:1, :, lo + half : hi],
            )

            def div_fin(qc=qc, onum=onum, bch=bch, osb=osb, lo=lo, hi=hi):
                nc.vector.tensor_mul(
                    out=osb[0:64, qc * QC + lo : qc * QC + hi],
                    in0=onum[0:64, 0, lo:hi],
                    in1=bch[:, 0, lo:hi],
                )
                nc.vector.tensor_mul(
                    out=osb[64:128, qc * QC + lo : qc * QC + hi],
                    in0=onum[0:64, 1, lo:hi],
                    in1=bch[:, 1, lo:hi],
                )

            if defer:
                lq.append(("df", div_fin))
            else:
                div_fin()

        def tail_recip(st, o_ps, lo, hi):
            """Tail divide, stage 1 for columns [lo:hi): lift the final
            ones-row off PSUM (plain shifted DVE copy), approximate
            reciprocal at partition 0, partition-broadcast.  Emitted as
            soon as the columns' last attn@V is in flight."""
            rd = smallp.tile([1, 2, QC], F32, name="rd", tag="rd")
            nc.vector.tensor_copy(
                out=rd[0:1, :, lo:hi], in_=o_ps[64:65, :, lo:hi]
            )
            nc.vector.reciprocal_approx_fast(
                out=rd[0:1, :, lo:hi], in_=rd[0:1, :, lo:hi]
            )
            bch = smallp.tile([64, 2, QC], F32, name="bch", tag="bch")
            nc.gpsimd.partition_broadcast(
                out_ap=bch[:, :, lo:hi], in_ap=rd[0:1, :, lo:hi]
            )
            return bch

        def tail_muls(st, o_ps, bch, qc, lo, hi):
            """Tail divide, stage 2: the muls read the numerator straight
            from PSUM and write both head halves of osb."""
            osb = st["osb"]
            nc.vector.tensor_mul(
                out=osb[0:64, qc * QC + lo : qc * QC + hi],
                in0=o_ps[0:64, 0, lo:hi],
                in1=bch[:, 0, lo:hi],
            )
            nc.vector.tensor_mul(
                out=osb[64:128, qc * QC + lo : qc * QC + hi],
                in0=o_ps[0:64, 1, lo:hi],
                in1=bch[:, 1, lo:hi],
            )

        for b in range(B):
            cur_b[0] = b
            if b == 0:
                state[0] = alloc_batch(0)
                us, _ = stage_a_units(state[0], 0, prologue=True)
                for i, u in enumerate(us):
                    u(evac="scalar" if i % 2 else "vector")
                    if i == 0:
                        nc.sync.dma_start(out=tri_sb[:, :], in_=tri[:, :])
                    elif i == 2:
                        # w1 right after the first qk unit is emitted: its
                        # transfer hides under that unit's matmuls
                        nc.scalar.dma_start(
                            out=w_sb[:, 1, :, :], in_=wqkv[1, :, :, :]
                        )
                    elif i == 4:
                        nc.scalar.dma_start(
                            out=w_sb[:, 2, :, :], in_=wqkv[2, :, :, :]
                        )
                        nc.sync.dma_start(out=wo_sb[:, :], in_=wo[:, :])
            if b + 1 < B:
                state[b + 1] = alloc_batch(b + 1)
                us, defer = stage_a_units(
                    state[b + 1], b + 1, defer_tail=(b + 1 == B - 1)
                )
                aq.extend(us)
                b3q.extend(defer)

            st = state[b]
            qt, ktt, vsb, osb = st["qt"], st["ktt"], st["vsb"], st["osb"]

            for qc in range(NQC):
                o_ps = psO.tile([65, 2, QC], F32, name="o_ps", tag="o")
                nkb = 4 * qc + 4
                last_qc = b == B - 1 and qc == NQC - 1

                def emit_av(kb, off, n, p_t, o_ps=o_ps, nkb=nkb):
                    for h in range(2):
                        nc.tensor.matmul(
                            o_ps[:, h, off:QC],
                            lhsT=vsb[:, kb, h, 0:65],
                            rhs=p_t[:, h, 0:n],
                            start=(kb == 0),
                            stop=(kb == nkb - 1),
                            skip_group_check=True,
                        )

                pending = []
                for kb in range(nkb):
                    off = max(0, (kb - 4 * qc) * 128)
                    n = QC - off
                    s_ps = psS.tile([128, 2, QC], F32, name="s_ps", tag="s")
                    for h in range(2):
                        nc.tensor.matmul(
                            s_ps[:, h, 0:n],
                            lhsT=ktt[
                                64 * h : 64 * h + 64,
                                kb * 128 : (kb + 1) * 128,
                            ],
                            rhs=qt[
                                64 * h : 64 * h + 64,
                                qc * QC + off : (qc + 1) * QC,
                            ],
                            start=True,
                            stop=True,
                        )
                    p_t = ptp.tile([128, 2, QC], F16, name="p_t", tag="pt")
                    if qc >= 2 and kb < 4 * qc and kb % 3 == 1:
                        # offload every 3rd full block's exp to the DVE
                        # (Schraudolph bitcast exp, ~2% element error) --
                        # the scalar engine paces these long query chunks
                        nc.vector.tensor_scalar(
                            out=p_t[:, :, 0:n].bitcast(I16),
                            in0=s_ps[:, :, 0:n],
                            scalar1=_EXP_C_MUL,
                            scalar2=_EXP_C_ADD,
                            op0=mybir.AluOpType.mult,
                            op1=mybir.AluOpType.add,
                        )
                    else:
                        nc.scalar.activation(
                            out=p_t[:, :, 0:n], in_=s_ps[:, :, 0:n], func=Exp
                        )
                    if kb >= 4 * qc:
                        nc.vector.tensor_mul(
                            out=p_t[:, :, 0:128],
                            in0=p_t[:, :, 0:128],
                            in1=tri_sb[:, :]
                            .unsqueeze(1)
                            .broadcast_to([128, 2, 128]),
                        )
                    # filler keeps the PE queue full while the exp for
                    # this block is still in flight
                    pop_filler(kb)
                    pending.append((kb, off, n, p_t))
                    if len(pending) > 3:
                        emit_av(*pending.pop(0))

                if qc >= 1:
                    for half in range(2):
                        for u in oproj_unit(st, b, qc - 1, half):
                            lq.append(("op", u))

                if not last_qc:
                    for pv in pending:
                        emit_av(*pv)
                    divide_chain(st, o_ps, qc, 0, QC)
                else:
                    # tail: pipeline the divide + output projection in two
                    # 256-query halves so the drain chain is half as long.
                    # queries [0:256) are final after the aV for kb13.
                    # queries [0:256) of this qc are final after kb13's
                    # attn@V, so its reciprocal chain runs while the PE
                    # streams kb14/kb15 (the cheap row-copy briefly blocks
                    # kb14's accumulate; the rest is off the PE's path)
                    emit_av(*pending.pop(0))          # kb13
                    bch0 = tail_recip(st, o_ps, 0, QC // 2)
                    for pv in pending:                # kb14, kb15
                        emit_av(*pv)
                    tail_muls(st, o_ps, bch0, qc, 0, QC // 2)
                    bch1 = tail_recip(st, o_ps, QC // 2, QC)
                    for u in oproj_unit(st, b, qc, 0, split_q=True):
                        u()
                    tail_muls(st, o_ps, bch1, qc, QC // 2, QC)
                    for u in oproj_unit(st, b, qc, 1, split_q=True):
                        u()

            # batch end: emit reserved projection units (they keep the PE
            # busy while the last divide chain completes) and carry the
            # remaining lq work into the next batch's slots.  Their PSUM
            # evacuations alternate scalar/vector so the banks recycle
            # even while the divide chain occupies the DVE queue.
            ei = 0
            while aq:
                aq.popleft()(evac="scalar" if ei % 2 else "vector")
                ei += 1
            last = b == B - 1
            if not last:
                for half in range(2):
                    for u in oproj_unit(st, b, NQC - 1, half):
                        lq.append(("op", u))
            else:
                while lq:
                    lq.popleft()[1]()
            if b - 1 in state:
                del state[b - 1]


def _build(use_bias=False):
    nc = bacc.Bacc("TRN2", target_bir_lowering=False)
    xt = nc.dram_tensor("xt", [128, R // RC, KT, RC], F16, kind="ExternalInput")
    wqkv = nc.dram_tensor("wqkv", [3, 128, KT, HD], F16, kind="ExternalInput")
    bqkv = nc.dram_tensor("bqkv", [3, HD], F32, kind="ExternalInput")
    wo = nc.dram_tensor("wo", [HD, C], F16, kind="ExternalInput")
    tri = nc.dram_tensor("tri", [128, 128], F16, kind="ExternalInput")
    y = nc.dram_tensor("y", [R, C], F16, kind="ExternalOutput")
    with tile.TileContext(nc) as tc:
        _emit(nc, tc, xt, wqkv, bqkv, wo, tri, y, use_bias)
    nc.finalize()
    return nc


def kernel(hidden_states, w_qkv, b_qkv, w_o, b_o):
    global LAST_RESULT, _CACHED_NC
    X = np.ascontiguousarray(np.asarray(hidden_states, dtype=np.float32)).reshape(
        R, C
    )
    w_qkv = np.asarray(w_qkv, dtype=np.float32)
    b_qkv = np.asarray(b_qkv, dtype=np.float32)
    w_o = np.asarray(w_o, dtype=np.float32)
    b_o = np.asarray(b_o, dtype=np.float32)

    # [ki, rc, ko, col] layout: each partition's per-chunk read is one
    # contiguous 8 KB run, so the x_t DMAs stream at full bandwidth
    Xt = X.T.astype(np.float16).reshape(KT, 128, R // RC, RC)
    Xt = np.ascontiguousarray(Xt.transpose(1, 2, 0, 3))
    scale = float(DH) ** -0.5
    tri_m = np.triu(np.ones((128, 128), dtype=np.float32)).astype(np.float16)

    in_maps = []
    for c in range(NCORES):
        heads = [HPC * c + i for i in range(HPC)]
        wcols, bcols = [], []
        for sec in range(3):  # q, k, v
            sc = scale if sec == 0 else 1.0
            for h in heads:
                lo = sec * C + h * DH
                wcols.append(w_qkv[:, lo : lo + DH] * sc)
                bcols.append(b_qkv[lo : lo + DH] * sc)
        wqkv_c = (
            np.concatenate(wcols, axis=1)
            .astype(np.float16)
            .reshape(KT, 128, 3, HD)
        )
        wqkv_c = np.ascontiguousarray(wqkv_c.transpose(2, 1, 0, 3))
        bqkv_c = np.ascontiguousarray(np.concatenate(bcols).reshape(3, HD))
        wo_c = np.ascontiguousarray(
            np.concatenate([w_o[h * DH : (h + 1) * DH, :] for h in heads], axis=0)
        ).astype(np.float16)  # [HD, C]
        in_maps.append(
            {
                "xt": Xt,
                "wqkv": wqkv_c,
                "bqkv": bqkv_c,
                "wo": wo_c,
                "tri": tri_m,
            }
        )

    if _CACHED_NC is None:
        _CACHED_NC = _build(use_bias=bool(np.any(b_qkv)))
    res = run_bass_kernel_spmd(_CACHED_NC, in_maps, core_ids=list(range(NCORES)))
    LAST_RESULT = res

    out = res.results[0]["y"].astype(np.float64)
    for c in range(1, NCORES):
        out += res.results[c]["y"]
    out += b_o
    return out.astype(np.float32).reshape(B, T, C)


# revision 22
# speedup vs baseline: 1.0271x; 1.0271x over previous
"""Causal multi-head attention block (qkv proj + attention + out proj) on 8
Trainium2 NeuronCores.

Sharding: Megatron-style tensor parallel over heads -- 2 heads per core.
Each core computes its heads' Q/K/V projections (column-sharded w_qkv),
causal attention for those heads, and a row-sharded partial of the output
projection.  The host sums the 8 partial outputs and adds b_o.

Device-side layout notes:
 - X^T [C, B*T] (fp16) feeds every matmul contraction dim on SBUF
   partitions with no on-device transposes.  Q^T/K^T come from the
   weight-stationary projection; V is produced keys-major directly by
   using the X^T tile as the stationary operand (out = X_chunk @ Wv), so
   no PE transpose or PSUM evacuation of V^T is needed.
 - Scores are computed transposed (S^T[k, q] = K^T.T @ Q^T per 128-wide
   k block) with the two heads' matmuls row-packed on the PE (partitions
   0:64 / 64:128).  Softmax exp runs on the scalar engine; the
   denominator is an extra all-ones column appended to V (row 64 of the
   attn@V accumulator).
 - The emission order interleaves the next batch's projections (aq) and
   deferred latency-sensitive work (lq: output projection, softmax
   divide finish) into the attention kb-loop so the PE queue never
   drains.  A reserve of projection units is held back to bridge the
   batch-boundary divide chain; the last batch's rcl2-3 projections are
   deferred into its own attention loop (it has no next batch to fill
   with).
 - Softmax divide: one evacuation copy frees the PSUM banks; the
   denominator reciprocal runs directly on the PSUM ones-row
   (reciprocal_approx_fast, fp32), is partition-broadcast in two halves,
   and the two muls are deferred (lq) so they never head-block the DVE
   queue.  Projection evacuations in prologue/batch-end bursts alternate
   between the scalar and vector engines so PSUM recycling is never
   serialized behind the divide chain.
 - The final query chunk of the last batch pipelines its divide + output
   projection in two 256-query halves to shorten the drain tail.
"""

import numpy as np
import ml_dtypes
from collections import deque
from contextlib import ExitStack

import concourse.bass as bass
import concourse.tile as tile
import concourse.mybir as mybir
from concourse import bacc
from concourse.bass_utils import run_bass_kernel_spmd

B, T, C, H, DH = 4, 2048, 1024, 16, 64
NCORES = 8
HPC = H // NCORES            # heads per core = 2
R = B * T                    # 8192 rows
HD = HPC * DH                # 128 local head dims
KT = C // 128                # 8 contraction tiles over C
RC = 512                     # row chunk in qkv stage
QC = 512                     # query chunk in attention
NQC = T // QC                # 4
NKB = T // 128               # 16 key blocks per batch

F32 = mybir.dt.float32
F16 = mybir.dt.float16
I16 = mybir.dt.int16

# Schraudolph fp16-bitcast exp constants (i16 = s*1024*log2(e) + bias)
_EXP_C_MUL = 1024.0 * 1.4426950408889634
_EXP_C_ADD = 1024.0 * 15.0 - 46.1277

LAST_RESULT = None           # BassKernelResults of the most recent run
_CACHED_NC = None


def _emit(nc, tc, xt, wqkv, bqkv, wo, tri, y, use_bias=False):
    Exp = mybir.ActivationFunctionType.Exp
    with ExitStack() as ctx:
        const = ctx.enter_context(tc.tile_pool(name="const", bufs=1))
        bigp = ctx.enter_context(tc.tile_pool(name="bigp", bufs=2))
        xtp = ctx.enter_context(tc.tile_pool(name="xtp", bufs=3))
        vsbp = ctx.enter_context(tc.tile_pool(name="vsbp", bufs=2))
        ptp = ctx.enter_context(tc.tile_pool(name="ptp", bufs=6))
        osbp = ctx.enter_context(tc.tile_pool(name="osbp", bufs=2))
        ystp = ctx.enter_context(tc.tile_pool(name="ystp", bufs=3))
        smallp = ctx.enter_context(tc.tile_pool(name="smallp", bufs=3))
        psP = ctx.enter_context(tc.tile_pool(name="psP", bufs=2, space="PSUM"))
        psS = ctx.enter_context(tc.tile_pool(name="psS", bufs=2, space="PSUM"))
        psO = ctx.enter_context(tc.tile_pool(name="psO", bufs=1, space="PSUM"))

        # ---- constants (issued on idle queues so the sync queue can
        # start streaming x_t immediately) ----
        w_sb = const.tile([128, 3, KT, HD], F16, name="w_sb")
        nc.scalar.dma_start(out=w_sb[:, 0, :, :], in_=wqkv[0, :, :, :])
        # wo/tri/bias loads are deferred behind the first x chunk on the
        # sync queue: they are not needed until the first oproj / diag
        # block, and issuing them at t0 steals DMA bandwidth from the
        # prologue-critical x stream
        wo_sb = const.tile([128, C], F16, name="wo_sb")
        tri_sb = const.tile([128, 128], F16, name="tri_sb")
        b_sb = const.tile([128, 3], F32, name="b_sb")
        if use_bias:
            for m in range(3):
                nc.gpsimd.dma_start(
                    out=b_sb[:, m : m + 1],
                    in_=bqkv[m : m + 1, :].rearrange("a n -> n a"),
                )
        bvv_sb = None
        if use_bias:
            bvv_sb = const.tile([128, HD], F32, name="bvv_sb")
            nc.sync.dma_start(
                out=bvv_sb[:, :],
                in_=bqkv[2:3, :].broadcast_to([128, HD]),
            )

        state = {}

        def alloc_batch(b):
            st = {
                "qt": bigp.tile([128, T], F16, name="qt", tag="qt"),
                "ktt": bigp.tile([128, T], F16, name="ktt", tag="ktt"),
                "vsb": vsbp.tile([128, NKB, 2, 65], F16, name="vsb", tag="vsb"),
                "osb": osbp.tile([128, T], F16, name="osb", tag="osb"),
                "xt": {},
            }
            return st

        def dma_unit(st, b2, rcl, halves=False, prologue=False):
            def f(evac="vector"):
                x_t = xtp.tile([128, KT, RC], F16, name="x_t", tag="xt")
                rc = b2 * (T // RC) + rcl
                eng = nc.sync
                if halves:
                    # subtile deps let the first matmuls start on the
                    # first quarters while the rest are still in flight
                    for kq in range(4):
                        eng.dma_start(
                            out=x_t[:, 2 * kq : 2 * kq + 2, :],
                            in_=xt[:, rc, 2 * kq : 2 * kq + 2, :],
                        )
                else:
                    eng.dma_start(out=x_t[:, :, :], in_=xt[:, rc, :, :])
                st["xt"][rcl] = x_t

            return f

        def ones_unit(st):
            def f(evac="vector"):
                nc.gpsimd.memset(st["vsb"][:, :, :, 64:65], 1.0)

            return f

        def qk_unit(st, rcl, m):
            def f(evac="vector"):
                x_t = st["xt"][rcl]
                ps = psP.tile([128, RC], F32, name="ps_qk", tag="pp")
                for k in range(KT):
                    nc.tensor.matmul(
                        ps[:, :],
                        lhsT=w_sb[:, m, k, :],
                        rhs=x_t[:, k, :],
                        start=(k == 0),
                        stop=(k == KT - 1),
                    )
                dst = (st["qt"] if m == 0 else st["ktt"])[
                    :, rcl * RC : (rcl + 1) * RC
                ]
                if use_bias:
                    nc.vector.tensor_scalar_add(
                        out=dst, in0=ps[:, :], scalar1=b_sb[:, m : m + 1]
                    )
                elif evac == "scalar":
                    nc.scalar.copy(out=dst, in_=ps[:, :])
                else:
                    nc.vector.tensor_copy(out=dst, in_=ps[:, :])

            return f

        def v_unit(st, rcl, rt):
            def f(evac="vector"):
                x_t = st["xt"][rcl]
                ps = psP.tile([128, RC], F32, name="ps_v", tag="pp")
                for k in range(KT):
                    nc.tensor.matmul(
                        ps[:, 0:128],
                        lhsT=x_t[:, k, rt * 128 : (rt + 1) * 128],
                        rhs=w_sb[:, 2, k, :],
                        start=(k == 0),
                        stop=(k == KT - 1),
                    )
                kb = rcl * 4 + rt
                vsb = st["vsb"]
                src = ps[:, 0:128].rearrange("p (h d) -> p h d", h=2)
                if use_bias:
                    # v bias varies along the free (dim) axis here, so a
                    # pre-replicated [128, HD] tile is added elementwise
                    nc.vector.tensor_add(
                        out=ps[:, 0:128], in0=ps[:, 0:128], in1=bvv_sb[:, :]
                    )
                    nc.vector.tensor_copy(out=vsb[:, kb, :, 0:64], in_=src)
                elif evac == "scalar":
                    nc.scalar.copy(out=vsb[:, kb, :, 0:64], in_=src)
                else:
                    nc.vector.tensor_copy(out=vsb[:, kb, :, 0:64], in_=src)

            return f

        def stage_a_units(st, b2, prologue=False, defer_tail=False):
            """Returns (units, deferred_units).  deferred_units (rcl 2-3
            projections) are only split out for the last batch, which has
            no successor to supply fillers for its attention loop."""
            defer = []
            if prologue:
                # first batch is gated on its own first chunk: don't put
                # prefetches ahead of it in the DMA engines
                us = [dma_unit(st, b2, 0, halves=True, prologue=True),
                      ones_unit(st)]
                for rcl in range(4):
                    us.append(qk_unit(st, rcl, 0))
                    if rcl + 1 < 4:
                        us.append(dma_unit(st, b2, rcl + 1, prologue=True))
                    us.append(qk_unit(st, rcl, 1))
                    for rt in range(4):
                        us.append(v_unit(st, rcl, rt))
                return us, defer
            us = [dma_unit(st, b2, 0), dma_unit(st, b2, 1), ones_unit(st)]
            for rcl in range(4):
                tgt = defer if (defer_tail and rcl >= 2) else us
                tgt.append(qk_unit(st, rcl, 0))
                if rcl + 2 < 4:
                    us.append(dma_unit(st, b2, rcl + 2))
                tgt.append(qk_unit(st, rcl, 1))
                for rt in range(4):
                    tgt.append(v_unit(st, rcl, rt))
            return us, defer

        def oproj_unit(st, b, qc, half, split_q=False):
            """Returns a list of two filler units (one per 128-row block)
            sharing one yst tile; the second unit issues the combined
            DMA.  Finer units spread PE filler work more evenly through
            the exp-paced diagonal regions."""
            osb = st["osb"]
            rb0 = 4 * qc + 2 * half
            yst = ystp.tile([128, 2, 2 * RC], F16, name="yst", tag="yst")

            def piece(i):
                def f(evac="vector"):
                    rb = rb0 + i
                    for j in range(2):
                        ps = psP.tile([128, 512], F32, name="ps_o", tag="pp")
                        nc.tensor.matmul(
                            ps[:, :],
                            lhsT=osb[:, rb * 128 : (rb + 1) * 128],
                            rhs=wo_sb[:, j * 512 : (j + 1) * 512],
                            start=True,
                            stop=True,
                        )
                        dst = yst[:, i, j * 512 : (j + 1) * 512]
                        if split_q and j == 1:
                            # drain tail: ACT is idle once the last exp
                            # is done, so split evacuation across engines
                            nc.scalar.copy(out=dst, in_=ps[:, :])
                        else:
                            nc.vector.tensor_copy(out=dst, in_=ps[:, :])
                    if split_q:
                        nc.sync.dma_start(
                            out=y[b * T + rb * 128 : b * T + (rb + 1) * 128, :],
                            in_=yst[:, i, :],
                        )
                    elif i == 1:
                        nc.sync.dma_start(
                            out=y[
                                b * T + rb0 * 128 : b * T + (rb0 + 2) * 128, :
                            ].rearrange("(i p) c -> p i c", i=2),
                            in_=yst[:, :, :],
                        )

                return f

            return [piece(0), piece(1)]

        # two filler streams woven into the attention kb-loop:
        #  - aq: next batch's projections (independent, always ready)
        #  - lq: latency-sensitive deferred work (softmax-divide finish,
        #    output projection) that must not reach an engine queue before
        #    its upstream chain has had time to complete
        #  - b3q: the last batch's own deferred rcl2-3 projections
        aq = deque()
        lq = deque()  # entries: (kind, fn); 'df' = divide-finish, 'op' = oproj
        b3q = deque()
        cur_b = [0]
        RESERVE = 2   # aq units held back to cushion the batch-end chain

        def pop_filler(slot):
            # the deferred divide-finish must not reach the DVE queue
            # until its reciprocal-broadcast inputs have landed (~2 slots)
            if lq and lq[0][0] == "df" and slot >= 2:
                lq.popleft()[1]()
            elif b3q and cur_b[0] == B - 1 and (slot % 2 == 0 or not lq):
                b3q.popleft()()
            elif lq and lq[0][0] != "df":
                lq.popleft()[1]()
            elif len(aq) > RESERVE:
                aq.popleft()()

        def divide_chain(st, o_ps, qc, lo, hi, defer=True):
            """Softmax divide for query columns [lo:hi) of this qc.
            Mid-batch: evacuate the attn@V accumulator (frees the PSUM
            banks), spread the ones-row across 32 lanes by DMA (sync
            queue -- it is idle), reciprocal there, despread, then
            partition-broadcast in two halves.  The two muls write both
            head halves of osb directly (engines shift partition bases
            on plain tensor ops, so the h1 mul writes partitions 64:128
            straight from base-0 inputs -- no shift DMA)."""
            osb = st["osb"]
            n = hi - lo
            if lo == 0:
                st["onum"] = smallp.tile([65, 2, QC], F32, name="onum", tag="on")
            onum = st["onum"]
            nc.vector.tensor_copy(
                out=onum[:, :, lo:hi], in_=o_ps[:, :, lo:hi]
            )
            sp = smallp.tile([32, QC // 8], F32, name="sp", tag="sp")
            nc.sync.dma_start(out=sp[:, 0 : n // 16],
                              in_=onum[64:65, :, lo:hi])
            sph = smallp.tile([32, QC // 8], F16, name="sph", tag="sph")
            with nc.allow_low_precision(
                reason="softmax reciprocal broadcast in fp16"
            ):
                nc.vector.reciprocal(
                    out=sph[:, 0 : n // 16], in_=sp[:, 0 : n // 16]
                )
            srow = smallp.tile([1, 2, QC], F16, name="srow", tag="srow")
            nc.sync.dma_start(out=srow[0:1, :, lo:hi],
                              in_=sph[:, 0 : n // 16])
            bch = smallp.tile([64, 2, QC], F16, name="bch", tag="bch")
            half = n // 2
            nc.gpsimd.partition_broadcast(
                out_ap=bch[:, :, lo : lo + half],
                in_ap=srow[0:1, :, lo : lo + half],
            )
            nc.gpsimd.partition_broadcast(
                out_ap=bch[:, :, lo + half : hi],
                in_ap=srow[0:1, :, lo + half : hi],
            )

            def div_fin(qc=qc, onum=onum, bch=bch, osb=osb, lo=lo, hi=hi):
                nc.vector.tensor_mul(
                    out=osb[0:64, qc * QC + lo : qc * QC + hi],
                    in0=onum[0:64, 0, lo:hi],
                    in1=bch[:, 0, lo:hi],
                )
                nc.vector.tensor_mul(
                    out=osb[64:128, qc * QC + lo : qc * QC + hi],
                    in0=onum[0:64, 1, lo:hi],
                    in1=bch[:, 1, lo:hi],
                )

            if defer:
                lq.append(("df", div_fin))
            else:
                div_fin()

        def tail_recip(st, o_ps, lo, hi):
            """Tail divide, stage 1 for columns [lo:hi): lift the final
            ones-row off PSUM (plain shifted DVE copy), approximate
            reciprocal at partition 0, partition-broadcast.  Emitted as
            soon as the columns' last attn@V is in flight."""
            rd = smallp.tile([1, 2, QC], F32, name="rd", tag="rd")
            nc.vector.tensor_copy(
                out=rd[0:1, :, lo:hi], in_=o_ps[64:65, :, lo:hi]
            )
            nc.vector.reciprocal_approx_fast(
                out=rd[0:1, :, lo:hi], in_=rd[0:1, :, lo:hi]
            )
            bch = smallp.tile([64, 2, QC], F32, name="bch", tag="bch")
            nc.gpsimd.partition_broadcast(
                out_ap=bch[:, :, lo:hi], in_ap=rd[0:1, :, lo:hi]
            )
            return bch

        def tail_muls(st, o_ps, bch, qc, lo, hi):
            """Tail divide, stage 2: the muls read the numerator straight
            from PSUM and write both head halves of osb."""
            osb = st["osb"]
            nc.vector.tensor_mul(
                out=osb[0:64, qc * QC + lo : qc * QC + hi],
                in0=o_ps[0:64, 0, lo:hi],
                in1=bch[:, 0, lo:hi],
            )
            nc.vector.tensor_mul(
                out=osb[64:128, qc * QC + lo : qc * QC + hi],
                in0=o_ps[0:64, 1, lo:hi],
                in1=bch[:, 1, lo:hi],
            )

        for b in range(B):
            cur_b[0] = b
            if b == 0:
                state[0] = alloc_batch(0)
                us, _ = stage_a_units(state[0], 0, prologue=True)
                for i, u in enumerate(us):
                    u(evac="scalar" if i % 2 else "vector")
                    if i == 0:
                        nc.sync.dma_start(out=tri_sb[:, :], in_=tri[:, :])
                    elif i == 2:
                        # w1 right after the first qk unit is emitted: its
                        # transfer hides under that unit's matmuls
                        nc.scalar.dma_start(
                            out=w_sb[:, 1, :, :], in_=wqkv[1, :, :, :]
                        )
                    elif i == 4:
                        nc.scalar.dma_start(
                            out=w_sb[:, 2, :, :], in_=wqkv[2, :, :, :]
                        )
                        nc.sync.dma_start(out=wo_sb[:, :], in_=wo[:, :])
            if b + 1 < B:
                state[b + 1] = alloc_batch(b + 1)
                us, defer = stage_a_units(
                    state[b + 1], b + 1, defer_tail=(b + 1 == B - 1)
                )
                aq.extend(us)
                b3q.extend(defer)

            st = state[b]
            qt, ktt, vsb, osb = st["qt"], st["ktt"], st["vsb"], st["osb"]

            for qc in range(NQC):
                o_ps = psO.tile([65, 2, QC], F32, name="o_ps", tag="o")
                nkb = 4 * qc + 4
                last_qc = b == B - 1 and qc == NQC - 1

                def emit_av(kb, off, n, p_t, o_ps=o_ps, nkb=nkb):
                    for h in range(2):
                        nc.tensor.matmul(
                            o_ps[:, h, off:QC],
                            lhsT=vsb[:, kb, h, 0:65],
                            rhs=p_t[:, h, 0:n],
                            start=(kb == 0),
                            stop=(kb == nkb - 1),
                            skip_group_check=True,
                        )

                pending = []
                for kb in range(nkb):
                    off = max(0, (kb - 4 * qc) * 128)
                    n = QC - off
                    s_ps = psS.tile([128, 2, QC], F32, name="s_ps", tag="s")
                    for h in range(2):
                        nc.tensor.matmul(
                            s_ps[:, h, 0:n],
                            lhsT=ktt[
                                64 * h : 64 * h + 64,
                                kb * 128 : (kb + 1) * 128,
                            ],
                            rhs=qt[
                                64 * h : 64 * h + 64,
                                qc * QC + off : (qc + 1) * QC,
                            ],
                            start=True,
                            stop=True,
                        )
                    p_t = ptp.tile([128, 2, QC], F16, name="p_t", tag="pt")
                    nc.scalar.activation(
                        out=p_t[:, :, 0:n], in_=s_ps[:, :, 0:n], func=Exp
                    )
                    if kb >= 4 * qc:
                        nc.vector.tensor_mul(
                            out=p_t[:, :, 0:128],
                            in0=p_t[:, :, 0:128],
                            in1=tri_sb[:, :]
                            .unsqueeze(1)
                            .broadcast_to([128, 2, 128]),
                        )
                    # filler keeps the PE queue full while the exp for
                    # this block is still in flight
                    pop_filler(kb)
                    pending.append((kb, off, n, p_t))
                    if len(pending) > 3:
                        emit_av(*pending.pop(0))

                if qc >= 1:
                    for half in range(2):
                        for u in oproj_unit(st, b, qc - 1, half):
                            lq.append(("op", u))

                if not last_qc:
                    for pv in pending:
                        emit_av(*pv)
                    divide_chain(st, o_ps, qc, 0, QC)
                else:
                    # tail: pipeline the divide + output projection in two
                    # 256-query halves so the drain chain is half as long.
                    # queries [0:256) are final after the aV for kb13.
                    # queries [0:256) of this qc are final after kb13's
                    # attn@V, so its reciprocal chain runs while the PE
                    # streams kb14/kb15 (the cheap row-copy briefly blocks
                    # kb14's accumulate; the rest is off the PE's path)
                    emit_av(*pending.pop(0))          # kb13
                    bch0 = tail_recip(st, o_ps, 0, QC // 2)
                    for pv in pending:                # kb14, kb15
                        emit_av(*pv)
                    tail_muls(st, o_ps, bch0, qc, 0, QC // 2)
                    bch1 = tail_recip(st, o_ps, QC // 2, QC)
                    for u in oproj_unit(st, b, qc, 0, split_q=True):
                        u()
                    tail_muls(st, o_ps, bch1, qc, QC // 2, QC)
                    for u in oproj_unit(st, b, qc, 1, split_q=True):
                        u()

            # batch end: emit reserved projection units (they keep the PE
            # busy while the last divide chain completes) and carry the
            # remaining lq work into the next batch's slots.  Their PSUM
            # evacuations alternate scalar/vector so the banks recycle
            # even while the divide chain occupies the DVE queue.
            ei = 0
            while aq:
                aq.popleft()(evac="scalar" if ei % 2 else "vector")
                ei += 1
            last = b == B - 1
            if not last:
                for half in range(2):
                    for u in oproj_unit(st, b, NQC - 1, half):
                        lq.append(("op", u))
            else:
                while lq:
                    lq.popleft()[1]()
            if b - 1 in state:
                del state[b - 1]


def _build(use_bias=False):
    nc = bacc.Bacc("TRN2", target_bir_lowering=False)
    xt = nc.dram_tensor("xt", [128, R // RC, KT, RC], F16, kind="ExternalInput")
    wqkv = nc.dram_tensor("wqkv", [3, 128, KT, HD], F16, kind="ExternalInput")
    bqkv = nc.dram_tensor("bqkv", [3, HD], F32, kind="ExternalInput")
    wo = nc.dram_tensor("wo", [HD, C], F16, kind="ExternalInput")
    tri = nc.dram_tensor("tri", [128, 128], F16, kind="ExternalInput")
    y = nc.dram_tensor("y", [R, C], F16, kind="ExternalOutput")
    with tile.TileContext(nc) as tc:
        _emit(nc, tc, xt, wqkv, bqkv, wo, tri, y, use_bias)
    nc.finalize()
    return nc


def kernel(hidden_states, w_qkv, b_qkv, w_o, b_o):
    global LAST_RESULT, _CACHED_NC
    X = np.ascontiguousarray(np.asarray(hidden_states, dtype=np.float32)).reshape(
        R, C
    )
    w_qkv = np.asarray(w_qkv, dtype=np.float32)
    b_qkv = np.asarray(b_qkv, dtype=np.float32)
    w_o = np.asarray(w_o, dtype=np.float32)
    b_o = np.asarray(b_o, dtype=np.float32)

    # [ki, rc, ko, col] layout: each partition's per-chunk read is one
    # contiguous 8 KB run, so the x_t DMAs stream at full bandwidth
    Xt = X.T.astype(np.float16).reshape(KT, 128, R // RC, RC)
    Xt = np.ascontiguousarray(Xt.transpose(1, 2, 0, 3))
    scale = float(DH) ** -0.5
    tri_m = np.triu(np.ones((128, 128), dtype=np.float32)).astype(np.float16)

    in_maps = []
    for c in range(NCORES):
        heads = [HPC * c + i for i in range(HPC)]
        wcols, bcols = [], []
        for sec in range(3):  # q, k, v
            sc = scale if sec == 0 else 1.0
            for h in heads:
                lo = sec * C + h * DH
                wcols.append(w_qkv[:, lo : lo + DH] * sc)
                bcols.append(b_qkv[lo : lo + DH] * sc)
        wqkv_c = (
            np.concatenate(wcols, axis=1)
            .astype(np.float16)
            .reshape(KT, 128, 3, HD)
        )
        wqkv_c = np.ascontiguousarray(wqkv_c.transpose(2, 1, 0, 3))
        bqkv_c = np.ascontiguousarray(np.concatenate(bcols).reshape(3, HD))
        wo_c = np.ascontiguousarray(
            np.concatenate([w_o[h * DH : (h + 1) * DH, :] for h in heads], axis=0)
        ).astype(np.float16)  # [HD, C]
        in_maps.append(
            {
                "xt": Xt,
                "wqkv": wqkv_c,
                "bqkv": bqkv_c,
                "wo": wo_c,
                "tri": tri_m,
            }
        )

    if _CACHED_NC is None:
        _CACHED_NC = _build(use_bias=bool(np.any(b_qkv)))
    res = run_bass_kernel_spmd(_CACHED_NC, in_maps, core_ids=list(range(NCORES)))
    LAST_RESULT = res

    out = res.results[0]["y"].astype(np.float64)
    for c in range(1, NCORES):
        out += res.results[c]["y"]
    out += b_o
    return out.astype(np.float32).reshape(B, T, C)


# revision 23
# speedup vs baseline: 1.0333x; 1.0060x over previous
"""Causal multi-head attention block (qkv proj + attention + out proj) on 8
Trainium2 NeuronCores.

Sharding: Megatron-style tensor parallel over heads -- 2 heads per core.
Each core computes its heads' Q/K/V projections (column-sharded w_qkv),
causal attention for those heads, and a row-sharded partial of the output
projection.  The host sums the 8 partial outputs and adds b_o.

Device-side layout notes:
 - X^T [C, B*T] (fp16) feeds every matmul contraction dim on SBUF
   partitions with no on-device transposes.  Q^T/K^T come from the
   weight-stationary projection; V is produced keys-major directly by
   using the X^T tile as the stationary operand (out = X_chunk @ Wv), so
   no PE transpose or PSUM evacuation of V^T is needed.
 - Scores are computed transposed (S^T[k, q] = K^T.T @ Q^T per 128-wide
   k block) with the two heads' matmuls row-packed on the PE (partitions
   0:64 / 64:128).  Softmax exp runs on the scalar engine; the
   denominator is an extra all-ones column appended to V (row 64 of the
   attn@V accumulator).
 - The emission order interleaves the next batch's projections (aq) and
   deferred latency-sensitive work (lq: output projection, softmax
   divide finish) into the attention kb-loop so the PE queue never
   drains.  A reserve of projection units is held back to bridge the
   batch-boundary divide chain; the last batch's rcl2-3 projections are
   deferred into its own attention loop (it has no next batch to fill
   with).
 - Softmax divide: one evacuation copy frees the PSUM banks; the
   denominator reciprocal runs directly on the PSUM ones-row
   (reciprocal_approx_fast, fp32), is partition-broadcast in two halves,
   and the two muls are deferred (lq) so they never head-block the DVE
   queue.  Projection evacuations in prologue/batch-end bursts alternate
   between the scalar and vector engines so PSUM recycling is never
   serialized behind the divide chain.
 - The final query chunk of the last batch pipelines its divide + output
   projection in two 256-query halves to shorten the drain tail.
"""

import numpy as np
import ml_dtypes
from collections import deque
from contextlib import ExitStack

import concourse.bass as bass
import concourse.tile as tile
import concourse.mybir as mybir
from concourse import bacc
from concourse.bass_utils import run_bass_kernel_spmd

B, T, C, H, DH = 4, 2048, 1024, 16, 64
NCORES = 8
HPC = H // NCORES            # heads per core = 2
R = B * T                    # 8192 rows
HD = HPC * DH                # 128 local head dims
KT = C // 128                # 8 contraction tiles over C
RC = 512                     # row chunk in qkv stage
QC = 512                     # query chunk in attention
NQC = T // QC                # 4
NKB = T // 128               # 16 key blocks per batch

F32 = mybir.dt.float32
F16 = mybir.dt.float16
I16 = mybir.dt.int16

# Schraudolph fp16-bitcast exp constants (i16 = s*1024*log2(e) + bias)
_EXP_C_MUL = 1024.0 * 1.4426950408889634
_EXP_C_ADD = 1024.0 * 15.0 - 46.1277

LAST_RESULT = None           # BassKernelResults of the most recent run
_CACHED_NC = None


def _emit(nc, tc, xt, wqkv, bqkv, wo, tri, y, use_bias=False):
    Exp = mybir.ActivationFunctionType.Exp
    with ExitStack() as ctx:
        const = ctx.enter_context(tc.tile_pool(name="const", bufs=1))
        bigp = ctx.enter_context(tc.tile_pool(name="bigp", bufs=2))
        xtp = ctx.enter_context(tc.tile_pool(name="xtp", bufs=3))
        vsbp = ctx.enter_context(tc.tile_pool(name="vsbp", bufs=2))
        ptp = ctx.enter_context(tc.tile_pool(name="ptp", bufs=6))
        osbp = ctx.enter_context(tc.tile_pool(name="osbp", bufs=2))
        ystp = ctx.enter_context(tc.tile_pool(name="ystp", bufs=3))
        smallp = ctx.enter_context(tc.tile_pool(name="smallp", bufs=3))
        psP = ctx.enter_context(tc.tile_pool(name="psP", bufs=2, space="PSUM"))
        psS = ctx.enter_context(tc.tile_pool(name="psS", bufs=2, space="PSUM"))
        psO = ctx.enter_context(tc.tile_pool(name="psO", bufs=1, space="PSUM"))

        # ---- constants (issued on idle queues so the sync queue can
        # start streaming x_t immediately) ----
        w_sb = const.tile([128, 3, KT, HD], F16, name="w_sb")
        nc.scalar.dma_start(out=w_sb[:, 0, :, :], in_=wqkv[0, :, :, :])
        # wo/tri/bias loads are deferred behind the first x chunk on the
        # sync queue: they are not needed until the first oproj / diag
        # block, and issuing them at t0 steals DMA bandwidth from the
        # prologue-critical x stream
        wo_sb = const.tile([128, C], F16, name="wo_sb")
        tri_sb = const.tile([128, 128], F16, name="tri_sb")
        b_sb = const.tile([128, 3], F32, name="b_sb")
        if use_bias:
            for m in range(3):
                nc.gpsimd.dma_start(
                    out=b_sb[:, m : m + 1],
                    in_=bqkv[m : m + 1, :].rearrange("a n -> n a"),
                )
        bvv_sb = None
        if use_bias:
            bvv_sb = const.tile([128, HD], F32, name="bvv_sb")
            nc.sync.dma_start(
                out=bvv_sb[:, :],
                in_=bqkv[2:3, :].broadcast_to([128, HD]),
            )

        state = {}

        def alloc_batch(b):
            st = {
                "qt": bigp.tile([128, T], F16, name="qt", tag="qt"),
                "ktt": bigp.tile([128, T], F16, name="ktt", tag="ktt"),
                "vsb": vsbp.tile([128, NKB, 2, 65], F16, name="vsb", tag="vsb"),
                "osb": osbp.tile([128, T], F16, name="osb", tag="osb"),
                "xt": {},
            }
            return st

        def dma_unit(st, b2, rcl, halves=False, prologue=False):
            def f(evac="vector"):
                x_t = xtp.tile([128, KT, RC], F16, name="x_t", tag="xt")
                rc = b2 * (T // RC) + rcl
                eng = nc.sync
                if halves:
                    # subtile deps let the first matmuls start on the
                    # first quarters while the rest are still in flight
                    for kq in range(4):
                        eng.dma_start(
                            out=x_t[:, 2 * kq : 2 * kq + 2, :],
                            in_=xt[:, rc, 2 * kq : 2 * kq + 2, :],
                        )
                else:
                    eng.dma_start(out=x_t[:, :, :], in_=xt[:, rc, :, :])
                st["xt"][rcl] = x_t

            return f

        def ones_unit(st):
            def f(evac="vector"):
                nc.gpsimd.memset(st["vsb"][:, :, :, 64:65], 1.0)

            return f

        def qk_unit(st, rcl, m):
            def f(evac="vector"):
                x_t = st["xt"][rcl]
                ps = psP.tile([128, RC], F32, name="ps_qk", tag="pp")
                for k in range(KT):
                    nc.tensor.matmul(
                        ps[:, :],
                        lhsT=w_sb[:, m, k, :],
                        rhs=x_t[:, k, :],
                        start=(k == 0),
                        stop=(k == KT - 1),
                    )
                dst = (st["qt"] if m == 0 else st["ktt"])[
                    :, rcl * RC : (rcl + 1) * RC
                ]
                if use_bias:
                    nc.vector.tensor_scalar_add(
                        out=dst, in0=ps[:, :], scalar1=b_sb[:, m : m + 1]
                    )
                elif evac == "scalar":
                    nc.scalar.copy(out=dst, in_=ps[:, :])
                else:
                    nc.vector.tensor_copy(out=dst, in_=ps[:, :])

            return f

        def v_unit(st, rcl, rt):
            def f(evac="vector"):
                x_t = st["xt"][rcl]
                ps = psP.tile([128, RC], F32, name="ps_v", tag="pp")
                for k in range(KT):
                    nc.tensor.matmul(
                        ps[:, 0:128],
                        lhsT=x_t[:, k, rt * 128 : (rt + 1) * 128],
                        rhs=w_sb[:, 2, k, :],
                        start=(k == 0),
                        stop=(k == KT - 1),
                    )
                kb = rcl * 4 + rt
                vsb = st["vsb"]
                src = ps[:, 0:128].rearrange("p (h d) -> p h d", h=2)
                if use_bias:
                    # v bias varies along the free (dim) axis here, so a
                    # pre-replicated [128, HD] tile is added elementwise
                    nc.vector.tensor_add(
                        out=ps[:, 0:128], in0=ps[:, 0:128], in1=bvv_sb[:, :]
                    )
                    nc.vector.tensor_copy(out=vsb[:, kb, :, 0:64], in_=src)
                elif evac == "scalar":
                    nc.scalar.copy(out=vsb[:, kb, :, 0:64], in_=src)
                else:
                    nc.vector.tensor_copy(out=vsb[:, kb, :, 0:64], in_=src)

            return f

        def stage_a_units(st, b2, prologue=False, defer_tail=False):
            """Returns (units, deferred_units).  deferred_units (rcl 2-3
            projections) are only split out for the last batch, which has
            no successor to supply fillers for its attention loop."""
            defer = []
            if prologue:
                # first batch is gated on its own first chunk: don't put
                # prefetches ahead of it in the DMA engines
                us = [dma_unit(st, b2, 0, halves=True, prologue=True),
                      ones_unit(st)]
                for rcl in range(4):
                    us.append(qk_unit(st, rcl, 0))
                    if rcl + 1 < 4:
                        us.append(dma_unit(st, b2, rcl + 1, prologue=True))
                    us.append(qk_unit(st, rcl, 1))
                    for rt in range(4):
                        us.append(v_unit(st, rcl, rt))
                return us, defer
            us = [dma_unit(st, b2, 0), dma_unit(st, b2, 1), ones_unit(st)]
            for rcl in range(4):
                tgt = defer if (defer_tail and rcl >= 2) else us
                tgt.append(qk_unit(st, rcl, 0))
                if rcl + 2 < 4:
                    us.append(dma_unit(st, b2, rcl + 2))
                tgt.append(qk_unit(st, rcl, 1))
                for rt in range(4):
                    tgt.append(v_unit(st, rcl, rt))
            return us, defer

        def oproj_unit(st, b, qc, half, split_q=False):
            """Returns a list of two filler units (one per 128-row block)
            sharing one yst tile; the second unit issues the combined
            DMA.  Finer units spread PE filler work more evenly through
            the exp-paced diagonal regions."""
            osb = st["osb"]
            rb0 = 4 * qc + 2 * half
            yst = ystp.tile([128, 2, 2 * RC], F16, name="yst", tag="yst")

            def piece(i):
                def f(evac="vector"):
                    rb = rb0 + i
                    for j in range(2):
                        ps = psP.tile([128, 512], F32, name="ps_o", tag="pp")
                        nc.tensor.matmul(
                            ps[:, :],
                            lhsT=osb[:, rb * 128 : (rb + 1) * 128],
                            rhs=wo_sb[:, j * 512 : (j + 1) * 512],
                            start=True,
                            stop=True,
                        )
                        dst = yst[:, i, j * 512 : (j + 1) * 512]
                        if split_q and j == 1:
                            # drain tail: ACT is idle once the last exp
                            # is done, so split evacuation across engines
                            nc.scalar.copy(out=dst, in_=ps[:, :])
                        else:
                            nc.vector.tensor_copy(out=dst, in_=ps[:, :])
                    if split_q:
                        nc.sync.dma_start(
                            out=y[b * T + rb * 128 : b * T + (rb + 1) * 128, :],
                            in_=yst[:, i, :],
                        )
                    elif i == 1:
                        nc.sync.dma_start(
                            out=y[
                                b * T + rb0 * 128 : b * T + (rb0 + 2) * 128, :
                            ].rearrange("(i p) c -> p i c", i=2),
                            in_=yst[:, :, :],
                        )

                return f

            return [piece(0), piece(1)]

        # two filler streams woven into the attention kb-loop:
        #  - aq: next batch's projections (independent, always ready)
        #  - lq: latency-sensitive deferred work (softmax-divide finish,
        #    output projection) that must not reach an engine queue before
        #    its upstream chain has had time to complete
        #  - b3q: the last batch's own deferred rcl2-3 projections
        aq = deque()
        lq = deque()  # entries: (kind, fn); 'df' = divide-finish, 'op' = oproj
        b3q = deque()
        cur_b = [0]
        RESERVE = 0   # aq units held back to cushion the batch-end chain

        def pop_filler(slot):
            # the deferred divide-finish must not reach the DVE queue
            # until its reciprocal-broadcast inputs have landed (~2 slots)
            if lq and lq[0][0] == "df" and slot >= 2:
                lq.popleft()[1]()
            elif b3q and cur_b[0] == B - 1 and (slot % 2 == 0 or not lq):
                b3q.popleft()()
            elif lq and lq[0][0] != "df":
                lq.popleft()[1]()
            elif len(aq) > RESERVE:
                aq.popleft()()

        def divide_chain(st, o_ps, qc, lo, hi, defer=True):
            """Softmax divide for query columns [lo:hi) of this qc.
            Mid-batch: evacuate the attn@V accumulator (frees the PSUM
            banks), spread the ones-row across 32 lanes by DMA (sync
            queue -- it is idle), reciprocal there, despread, then
            partition-broadcast in two halves.  The two muls write both
            head halves of osb directly (engines shift partition bases
            on plain tensor ops, so the h1 mul writes partitions 64:128
            straight from base-0 inputs -- no shift DMA)."""
            osb = st["osb"]
            n = hi - lo
            if lo == 0:
                st["onum"] = smallp.tile([65, 2, QC], F32, name="onum", tag="on")
            onum = st["onum"]
            nc.vector.tensor_copy(
                out=onum[:, :, lo:hi], in_=o_ps[:, :, lo:hi]
            )
            sp = smallp.tile([32, QC // 8], F32, name="sp", tag="sp")
            nc.sync.dma_start(out=sp[:, 0 : n // 16],
                              in_=onum[64:65, :, lo:hi])
            sph = smallp.tile([32, QC // 8], F16, name="sph", tag="sph")
            with nc.allow_low_precision(
                reason="softmax reciprocal broadcast in fp16"
            ):
                nc.vector.reciprocal(
                    out=sph[:, 0 : n // 16], in_=sp[:, 0 : n // 16]
                )
            srow = smallp.tile([1, 2, QC], F16, name="srow", tag="srow")
            nc.sync.dma_start(out=srow[0:1, :, lo:hi],
                              in_=sph[:, 0 : n // 16])
            bch = smallp.tile([64, 2, QC], F16, name="bch", tag="bch")
            half = n // 2
            nc.gpsimd.partition_broadcast(
                out_ap=bch[:, :, lo : lo + half],
                in_ap=srow[0:1, :, lo : lo + half],
            )
            nc.gpsimd.partition_broadcast(
                out_ap=bch[:, :, lo + half : hi],
                in_ap=srow[0:1, :, lo + half : hi],
            )

            def div_fin(qc=qc, onum=onum, bch=bch, osb=osb, lo=lo, hi=hi):
                nc.vector.tensor_mul(
                    out=osb[0:64, qc * QC + lo : qc * QC + hi],
                    in0=onum[0:64, 0, lo:hi],
                    in1=bch[:, 0, lo:hi],
                )
                nc.vector.tensor_mul(
                    out=osb[64:128, qc * QC + lo : qc * QC + hi],
                    in0=onum[0:64, 1, lo:hi],
                    in1=bch[:, 1, lo:hi],
                )

            if defer:
                lq.append(("df", div_fin))
            else:
                div_fin()

        def tail_recip(st, o_ps, lo, hi):
            """Tail divide, stage 1 for columns [lo:hi): lift the final
            ones-row off PSUM (plain shifted DVE copy), approximate
            reciprocal at partition 0, partition-broadcast.  Emitted as
            soon as the columns' last attn@V is in flight."""
            rd = smallp.tile([1, 2, QC], F32, name="rd", tag="rd")
            nc.vector.tensor_copy(
                out=rd[0:1, :, lo:hi], in_=o_ps[64:65, :, lo:hi]
            )
            nc.vector.reciprocal_approx_fast(
                out=rd[0:1, :, lo:hi], in_=rd[0:1, :, lo:hi]
            )
            bch = smallp.tile([64, 2, QC], F32, name="bch", tag="bch")
            nc.gpsimd.partition_broadcast(
                out_ap=bch[:, :, lo:hi], in_ap=rd[0:1, :, lo:hi]
            )
            return bch

        def tail_muls(st, o_ps, bch, qc, lo, hi):
            """Tail divide, stage 2: the muls read the numerator straight
            from PSUM and write both head halves of osb."""
            osb = st["osb"]
            nc.vector.tensor_mul(
                out=osb[0:64, qc * QC + lo : qc * QC + hi],
                in0=o_ps[0:64, 0, lo:hi],
                in1=bch[:, 0, lo:hi],
            )
            nc.vector.tensor_mul(
                out=osb[64:128, qc * QC + lo : qc * QC + hi],
                in0=o_ps[0:64, 1, lo:hi],
                in1=bch[:, 1, lo:hi],
            )

        for b in range(B):
            cur_b[0] = b
            if b == 0:
                state[0] = alloc_batch(0)
                us, _ = stage_a_units(state[0], 0, prologue=True)
                for i, u in enumerate(us):
                    u(evac="scalar" if i % 2 else "vector")
                    if i == 0:
                        nc.sync.dma_start(out=tri_sb[:, :], in_=tri[:, :])
                    elif i == 2:
                        # w1 right after the first qk unit is emitted: its
                        # transfer hides under that unit's matmuls
                        nc.scalar.dma_start(
                            out=w_sb[:, 1, :, :], in_=wqkv[1, :, :, :]
                        )
                    elif i == 4:
                        nc.scalar.dma_start(
                            out=w_sb[:, 2, :, :], in_=wqkv[2, :, :, :]
                        )
                        nc.sync.dma_start(out=wo_sb[:, :], in_=wo[:, :])
            if b + 1 < B:
                state[b + 1] = alloc_batch(b + 1)
                us, defer = stage_a_units(
                    state[b + 1], b + 1, defer_tail=(b + 1 == B - 1)
                )
                aq.extend(us)
                b3q.extend(defer)

            st = state[b]
            qt, ktt, vsb, osb = st["qt"], st["ktt"], st["vsb"], st["osb"]

            for qc in range(NQC):
                o_ps = psO.tile([65, 2, QC], F32, name="o_ps", tag="o")
                nkb = 4 * qc + 4
                last_qc = b == B - 1 and qc == NQC - 1

                def emit_av(kb, off, n, p_t, o_ps=o_ps, nkb=nkb):
                    for h in range(2):
                        nc.tensor.matmul(
                            o_ps[:, h, off:QC],
                            lhsT=vsb[:, kb, h, 0:65],
                            rhs=p_t[:, h, 0:n],
                            start=(kb == 0),
                            stop=(kb == nkb - 1),
                            skip_group_check=True,
                        )

                pending = []
                for kb in range(nkb):
                    off = max(0, (kb - 4 * qc) * 128)
                    n = QC - off
                    s_ps = psS.tile([128, 2, QC], F32, name="s_ps", tag="s")
                    for h in range(2):
                        nc.tensor.matmul(
                            s_ps[:, h, 0:n],
                            lhsT=ktt[
                                64 * h : 64 * h + 64,
                                kb * 128 : (kb + 1) * 128,
                            ],
                            rhs=qt[
                                64 * h : 64 * h + 64,
                                qc * QC + off : (qc + 1) * QC,
                            ],
                            start=True,
                            stop=True,
                        )
                    p_t = ptp.tile([128, 2, QC], F16, name="p_t", tag="pt")
                    nc.scalar.activation(
                        out=p_t[:, :, 0:n], in_=s_ps[:, :, 0:n], func=Exp
                    )
                    if kb >= 4 * qc:
                        nc.vector.tensor_mul(
                            out=p_t[:, :, 0:128],
                            in0=p_t[:, :, 0:128],
                            in1=tri_sb[:, :]
                            .unsqueeze(1)
                            .broadcast_to([128, 2, 128]),
                        )
                    # filler keeps the PE queue full while the exp for
                    # this block is still in flight
                    pop_filler(kb)
                    pending.append((kb, off, n, p_t))
                    if len(pending) > 3:
                        emit_av(*pending.pop(0))

                if qc >= 1:
                    for half in range(2):
                        for u in oproj_unit(st, b, qc - 1, half):
                            lq.append(("op", u))

                if not last_qc:
                    for pv in pending:
                        emit_av(*pv)
                    divide_chain(st, o_ps, qc, 0, QC)
                else:
                    # tail: pipeline the divide + output projection in two
                    # 256-query halves so the drain chain is half as long.
                    # queries [0:256) are final after the aV for kb13.
                    # queries [0:256) of this qc are final after kb13's
                    # attn@V, so its reciprocal chain runs while the PE
                    # streams kb14/kb15 (the cheap row-copy briefly blocks
                    # kb14's accumulate; the rest is off the PE's path)
                    emit_av(*pending.pop(0))          # kb13
                    bch0 = tail_recip(st, o_ps, 0, QC // 2)
                    for pv in pending:                # kb14, kb15
                        emit_av(*pv)
                    tail_muls(st, o_ps, bch0, qc, 0, QC // 2)
                    bch1 = tail_recip(st, o_ps, QC // 2, QC)
                    for u in oproj_unit(st, b, qc, 0, split_q=True):
                        u()
                    tail_muls(st, o_ps, bch1, qc, QC // 2, QC)
                    for u in oproj_unit(st, b, qc, 1, split_q=True):
                        u()

            # batch end: emit reserved projection units (they keep the PE
            # busy while the last divide chain completes) and carry the
            # remaining lq work into the next batch's slots.  Their PSUM
            # evacuations alternate scalar/vector so the banks recycle
            # even while the divide chain occupies the DVE queue.
            ei = 0
            while aq:
                aq.popleft()(evac="scalar" if ei % 2 else "vector")
                ei += 1
            last = b == B - 1
            if not last:
                for half in range(2):
                    for u in oproj_unit(st, b, NQC - 1, half):
                        lq.append(("op", u))
            else:
                while lq:
                    lq.popleft()[1]()
            if b - 1 in state:
                del state[b - 1]


def _build(use_bias=False):
    nc = bacc.Bacc("TRN2", target_bir_lowering=False)
    xt = nc.dram_tensor("xt", [128, R // RC, KT, RC], F16, kind="ExternalInput")
    wqkv = nc.dram_tensor("wqkv", [3, 128, KT, HD], F16, kind="ExternalInput")
    bqkv = nc.dram_tensor("bqkv", [3, HD], F32, kind="ExternalInput")
    wo = nc.dram_tensor("wo", [HD, C], F16, kind="ExternalInput")
    tri = nc.dram_tensor("tri", [128, 128], F16, kind="ExternalInput")
    y = nc.dram_tensor("y", [R, C], F16, kind="ExternalOutput")
    with tile.TileContext(nc) as tc:
        _emit(nc, tc, xt, wqkv, bqkv, wo, tri, y, use_bias)
    nc.finalize()
    return nc


def kernel(hidden_states, w_qkv, b_qkv, w_o, b_o):
    global LAST_RESULT, _CACHED_NC
    X = np.ascontiguousarray(np.asarray(hidden_states, dtype=np.float32)).reshape(
        R, C
    )
    w_qkv = np.asarray(w_qkv, dtype=np.float32)
    b_qkv = np.asarray(b_qkv, dtype=np.float32)
    w_o = np.asarray(w_o, dtype=np.float32)
    b_o = np.asarray(b_o, dtype=np.float32)

    # [ki, rc, ko, col] layout: each partition's per-chunk read is one
    # contiguous 8 KB run, so the x_t DMAs stream at full bandwidth
    Xt = X.T.astype(np.float16).reshape(KT, 128, R // RC, RC)
    Xt = np.ascontiguousarray(Xt.transpose(1, 2, 0, 3))
    scale = float(DH) ** -0.5
    tri_m = np.triu(np.ones((128, 128), dtype=np.float32)).astype(np.float16)

    in_maps = []
    for c in range(NCORES):
        heads = [HPC * c + i for i in range(HPC)]
        wcols, bcols = [], []
        for sec in range(3):  # q, k, v
            sc = scale if sec == 0 else 1.0
            for h in heads:
                lo = sec * C + h * DH
                wcols.append(w_qkv[:, lo : lo + DH] * sc)
                bcols.append(b_qkv[lo : lo + DH] * sc)
        wqkv_c = (
            np.concatenate(wcols, axis=1)
            .astype(np.float16)
            .reshape(KT, 128, 3, HD)
        )
        wqkv_c = np.ascontiguousarray(wqkv_c.transpose(2, 1, 0, 3))
        bqkv_c = np.ascontiguousarray(np.concatenate(bcols).reshape(3, HD))
        wo_c = np.ascontiguousarray(
            np.concatenate([w_o[h * DH : (h + 1) * DH, :] for h in heads], axis=0)
        ).astype(np.float16)  # [HD, C]
        in_maps.append(
            {
                "xt": Xt,
                "wqkv": wqkv_c,
                "bqkv": bqkv_c,
                "wo": wo_c,
                "tri": tri_m,
            }
        )

    if _CACHED_NC is None:
        _CACHED_NC = _build(use_bias=bool(np.any(b_qkv)))
    res = run_bass_kernel_spmd(_CACHED_NC, in_maps, core_ids=list(range(NCORES)))
    LAST_RESULT = res

    out = res.results[0]["y"].astype(np.float64)
    for c in range(1, NCORES):
        out += res.results[c]["y"]
    out += b_o
    return out.astype(np.float32).reshape(B, T, C)


# revision 24
# speedup vs baseline: 1.0404x; 1.0068x over previous
"""Causal multi-head attention block (qkv proj + attention + out proj) on 8
Trainium2 NeuronCores.

Sharding: Megatron-style tensor parallel over heads -- 2 heads per core.
Each core computes its heads' Q/K/V projections (column-sharded w_qkv),
causal attention for those heads, and a row-sharded partial of the output
projection.  The host sums the 8 partial outputs and adds b_o.

Device-side layout notes:
 - X^T [C, B*T] (fp16) feeds every matmul contraction dim on SBUF
   partitions with no on-device transposes.  Q^T/K^T come from the
   weight-stationary projection; V is produced keys-major directly by
   using the X^T tile as the stationary operand (out = X_chunk @ Wv), so
   no PE transpose or PSUM evacuation of V^T is needed.
 - Scores are computed transposed (S^T[k, q] = K^T.T @ Q^T per 128-wide
   k block) with the two heads' matmuls row-packed on the PE (partitions
   0:64 / 64:128).  Softmax exp runs on the scalar engine; the
   denominator is an extra all-ones column appended to V (row 64 of the
   attn@V accumulator).
 - The emission order interleaves the next batch's projections (aq) and
   deferred latency-sensitive work (lq: output projection, softmax
   divide finish) into the attention kb-loop so the PE queue never
   drains.  A reserve of projection units is held back to bridge the
   batch-boundary divide chain; the last batch's rcl2-3 projections are
   deferred into its own attention loop (it has no next batch to fill
   with).
 - Softmax divide: one evacuation copy frees the PSUM banks; the
   denominator ones-row is lane-spread by DMA (sync queue), reciprocaled
   on the DVE, despread and partition-broadcast in two halves; the two
   muls write both head halves of osb directly (engines shift partition
   bases on plain tensor ops, so no shift DMA) and are deferred (lq) so
   they never head-block the DVE queue.  Projection evacuations in
   prologue/batch-end bursts alternate between the scalar and vector
   engines so PSUM recycling is never serialized behind the divide chain.
 - The drain tail (last batch's final query chunk) uses a shorter
   DVE-only divide: shifted row copy off PSUM + reciprocal_approx_fast +
   broadcast, pipelined in two 256-query halves interleaved with the
   last attn@V accumulates, with numerator muls reading PSUM directly.
"""

import numpy as np
import ml_dtypes
from collections import deque
from contextlib import ExitStack

import concourse.bass as bass
import concourse.tile as tile
import concourse.mybir as mybir
from concourse import bacc
from concourse.bass_utils import run_bass_kernel_spmd

B, T, C, H, DH = 4, 2048, 1024, 16, 64
NCORES = 8
HPC = H // NCORES            # heads per core = 2
R = B * T                    # 8192 rows
HD = HPC * DH                # 128 local head dims
KT = C // 128                # 8 contraction tiles over C
RC = 512                     # row chunk in qkv stage
QC = 512                     # query chunk in attention
NQC = T // QC                # 4
NKB = T // 128               # 16 key blocks per batch

F32 = mybir.dt.float32
F16 = mybir.dt.float16
I16 = mybir.dt.int16

# Schraudolph fp16-bitcast exp constants (i16 = s*1024*log2(e) + bias)
_EXP_C_MUL = 1024.0 * 1.4426950408889634
_EXP_C_ADD = 1024.0 * 15.0 - 46.1277

LAST_RESULT = None           # BassKernelResults of the most recent run
_CACHED_NC = None


def _emit(nc, tc, xt, wqkv, bqkv, wo, tri, y, use_bias=False):
    Exp = mybir.ActivationFunctionType.Exp
    with ExitStack() as ctx:
        const = ctx.enter_context(tc.tile_pool(name="const", bufs=1))
        bigp = ctx.enter_context(tc.tile_pool(name="bigp", bufs=2))
        xtp = ctx.enter_context(tc.tile_pool(name="xtp", bufs=3))
        vsbp = ctx.enter_context(tc.tile_pool(name="vsbp", bufs=2))
        ptp = ctx.enter_context(tc.tile_pool(name="ptp", bufs=6))
        osbp = ctx.enter_context(tc.tile_pool(name="osbp", bufs=2))
        ystp = ctx.enter_context(tc.tile_pool(name="ystp", bufs=3))
        smallp = ctx.enter_context(tc.tile_pool(name="smallp", bufs=3))
        psP = ctx.enter_context(tc.tile_pool(name="psP", bufs=2, space="PSUM"))
        psS = ctx.enter_context(tc.tile_pool(name="psS", bufs=2, space="PSUM"))
        psO = ctx.enter_context(tc.tile_pool(name="psO", bufs=1, space="PSUM"))

        # ---- constants (issued on idle queues so the sync queue can
        # start streaming x_t immediately) ----
        w_sb = const.tile([128, 3, KT, HD], F16, name="w_sb")
        nc.scalar.dma_start(out=w_sb[:, 0, :, :], in_=wqkv[0, :, :, :])
        # wo/tri/bias loads are deferred behind the first x chunk on the
        # sync queue: they are not needed until the first oproj / diag
        # block, and issuing them at t0 steals DMA bandwidth from the
        # prologue-critical x stream
        wo_sb = const.tile([128, C], F16, name="wo_sb")
        tri_sb = const.tile([128, 128], F16, name="tri_sb")
        b_sb = const.tile([128, 3], F32, name="b_sb")
        if use_bias:
            for m in range(3):
                nc.gpsimd.dma_start(
                    out=b_sb[:, m : m + 1],
                    in_=bqkv[m : m + 1, :].rearrange("a n -> n a"),
                )
        bvv_sb = None
        if use_bias:
            bvv_sb = const.tile([128, HD], F32, name="bvv_sb")
            nc.sync.dma_start(
                out=bvv_sb[:, :],
                in_=bqkv[2:3, :].broadcast_to([128, HD]),
            )

        state = {}

        def alloc_batch(b):
            st = {
                "qt": bigp.tile([128, T], F16, name="qt", tag="qt"),
                "ktt": bigp.tile([128, T], F16, name="ktt", tag="ktt"),
                "vsb": vsbp.tile([128, NKB, 2, 65], F16, name="vsb", tag="vsb"),
                "osb": osbp.tile([128, T], F16, name="osb", tag="osb"),
                "xt": {},
            }
            return st

        def dma_unit(st, b2, rcl, halves=False, prologue=False):
            def f(evac="vector"):
                x_t = xtp.tile([128, KT, RC], F16, name="x_t", tag="xt")
                rc = b2 * (T // RC) + rcl
                eng = nc.sync
                if halves:
                    # subtile deps let the first matmuls start on the
                    # first quarters while the rest are still in flight
                    for kq in range(4):
                        eng.dma_start(
                            out=x_t[:, 2 * kq : 2 * kq + 2, :],
                            in_=xt[:, rc, 2 * kq : 2 * kq + 2, :],
                        )
                else:
                    eng.dma_start(out=x_t[:, :, :], in_=xt[:, rc, :, :])
                st["xt"][rcl] = x_t

            return f

        def ones_unit(st):
            def f(evac="vector"):
                nc.gpsimd.memset(st["vsb"][:, :, :, 64:65], 1.0)

            return f

        def qk_unit(st, rcl, m):
            def f(evac="vector"):
                x_t = st["xt"][rcl]
                ps = psP.tile([128, RC], F32, name="ps_qk", tag="pp")
                for k in range(KT):
                    nc.tensor.matmul(
                        ps[:, :],
                        lhsT=w_sb[:, m, k, :],
                        rhs=x_t[:, k, :],
                        start=(k == 0),
                        stop=(k == KT - 1),
                    )
                dst = (st["qt"] if m == 0 else st["ktt"])[
                    :, rcl * RC : (rcl + 1) * RC
                ]
                if use_bias:
                    nc.vector.tensor_scalar_add(
                        out=dst, in0=ps[:, :], scalar1=b_sb[:, m : m + 1]
                    )
                elif evac == "scalar":
                    nc.scalar.copy(out=dst, in_=ps[:, :])
                else:
                    nc.vector.tensor_copy(out=dst, in_=ps[:, :])

            return f

        def v_unit(st, rcl, rt):
            def f(evac="vector"):
                x_t = st["xt"][rcl]
                ps = psP.tile([128, RC], F32, name="ps_v", tag="pp")
                for k in range(KT):
                    nc.tensor.matmul(
                        ps[:, 0:128],
                        lhsT=x_t[:, k, rt * 128 : (rt + 1) * 128],
                        rhs=w_sb[:, 2, k, :],
                        start=(k == 0),
                        stop=(k == KT - 1),
                    )
                kb = rcl * 4 + rt
                vsb = st["vsb"]
                src = ps[:, 0:128].rearrange("p (h d) -> p h d", h=2)
                if use_bias:
                    # v bias varies along the free (dim) axis here, so a
                    # pre-replicated [128, HD] tile is added elementwise
                    nc.vector.tensor_add(
                        out=ps[:, 0:128], in0=ps[:, 0:128], in1=bvv_sb[:, :]
                    )
                    nc.vector.tensor_copy(out=vsb[:, kb, :, 0:64], in_=src)
                elif evac == "scalar":
                    nc.scalar.copy(out=vsb[:, kb, :, 0:64], in_=src)
                else:
                    nc.vector.tensor_copy(out=vsb[:, kb, :, 0:64], in_=src)

            return f

        def stage_a_units(st, b2, prologue=False, defer_tail=False):
            """Returns (units, deferred_units).  deferred_units (rcl 2-3
            projections) are only split out for the last batch, which has
            no successor to supply fillers for its attention loop."""
            defer = []
            if prologue:
                # first batch is gated on its own first chunk: don't put
                # prefetches ahead of it in the DMA engines
                us = [dma_unit(st, b2, 0, halves=True, prologue=True),
                      ones_unit(st)]
                for rcl in range(4):
                    us.append(qk_unit(st, rcl, 0))
                    if rcl + 1 < 4:
                        us.append(dma_unit(st, b2, rcl + 1, prologue=True))
                    us.append(qk_unit(st, rcl, 1))
                    for rt in range(4):
                        us.append(v_unit(st, rcl, rt))
                return us, defer
            us = [dma_unit(st, b2, 0), dma_unit(st, b2, 1), ones_unit(st)]
            for rcl in range(4):
                tgt = defer if (defer_tail and rcl >= 2) else us
                tgt.append(qk_unit(st, rcl, 0))
                if rcl + 2 < 4:
                    us.append(dma_unit(st, b2, rcl + 2))
                tgt.append(qk_unit(st, rcl, 1))
                for rt in range(4):
                    tgt.append(v_unit(st, rcl, rt))
            return us, defer

        def oproj_unit(st, b, qc, half, split_q=False):
            """Returns a list of two filler units (one per 128-row block)
            sharing one yst tile; the second unit issues the combined
            DMA.  Finer units spread PE filler work more evenly through
            the exp-paced diagonal regions."""
            osb = st["osb"]
            rb0 = 4 * qc + 2 * half
            yst = ystp.tile([128, 2, 2 * RC], F16, name="yst", tag="yst")

            def piece(i):
                def f(evac="vector"):
                    rb = rb0 + i
                    for j in range(2):
                        ps = psP.tile([128, 512], F32, name="ps_o", tag="pp")
                        nc.tensor.matmul(
                            ps[:, :],
                            lhsT=osb[:, rb * 128 : (rb + 1) * 128],
                            rhs=wo_sb[:, j * 512 : (j + 1) * 512],
                            start=True,
                            stop=True,
                        )
                        dst = yst[:, i, j * 512 : (j + 1) * 512]
                        if split_q and j == 1:
                            # drain tail: ACT is idle once the last exp
                            # is done, so split evacuation across engines
                            nc.scalar.copy(out=dst, in_=ps[:, :])
                        else:
                            nc.vector.tensor_copy(out=dst, in_=ps[:, :])
                    if split_q:
                        nc.sync.dma_start(
                            out=y[b * T + rb * 128 : b * T + (rb + 1) * 128, :],
                            in_=yst[:, i, :],
                        )
                    elif i == 1:
                        nc.sync.dma_start(
                            out=y[
                                b * T + rb0 * 128 : b * T + (rb0 + 2) * 128, :
                            ].rearrange("(i p) c -> p i c", i=2),
                            in_=yst[:, :, :],
                        )

                return f

            return [piece(0), piece(1)]

        # two filler streams woven into the attention kb-loop:
        #  - aq: next batch's projections (independent, always ready)
        #  - lq: latency-sensitive deferred work (softmax-divide finish,
        #    output projection) that must not reach an engine queue before
        #    its upstream chain has had time to complete
        #  - b3q: the last batch's own deferred rcl2-3 projections
        aq = deque()
        lq = deque()  # entries: (kind, fn); 'df' = divide-finish, 'op' = oproj
        b3q = deque()
        cur_b = [0]
        RESERVE = 0   # aq units held back to cushion the batch-end chain

        def pop_filler(slot):
            # the deferred divide-finish must not reach the DVE queue
            # until its reciprocal-broadcast inputs have landed (~2 slots)
            if lq and lq[0][0] == "df" and slot >= 2:
                lq.popleft()[1]()
            elif b3q and cur_b[0] == B - 1 and (slot % 2 == 0 or not lq):
                b3q.popleft()()
            elif lq and lq[0][0] != "df":
                lq.popleft()[1]()
            elif len(aq) > RESERVE:
                aq.popleft()()

        def divide_chain(st, o_ps, qc, lo, hi, defer=True):
            """Softmax divide for query columns [lo:hi) of this qc.
            Mid-batch: evacuate the attn@V accumulator (frees the PSUM
            banks), spread the ones-row across 32 lanes by DMA (sync
            queue -- it is idle), reciprocal there, despread, then
            partition-broadcast in two halves.  The two muls write both
            head halves of osb directly (engines shift partition bases
            on plain tensor ops, so the h1 mul writes partitions 64:128
            straight from base-0 inputs -- no shift DMA)."""
            osb = st["osb"]
            n = hi - lo
            if lo == 0:
                st["onum"] = smallp.tile([65, 2, QC], F32, name="onum", tag="on")
            onum = st["onum"]
            nc.vector.tensor_copy(
                out=onum[:, :, lo:hi], in_=o_ps[:, :, lo:hi]
            )
            sp = smallp.tile([32, QC // 8], F32, name="sp", tag="sp")
            nc.sync.dma_start(out=sp[:, 0 : n // 16],
                              in_=onum[64:65, :, lo:hi])
            sph = smallp.tile([32, QC // 8], F16, name="sph", tag="sph")
            with nc.allow_low_precision(
                reason="softmax reciprocal broadcast in fp16"
            ):
                nc.vector.reciprocal(
                    out=sph[:, 0 : n // 16], in_=sp[:, 0 : n // 16]
                )
            srow = smallp.tile([1, 2, QC], F16, name="srow", tag="srow")
            nc.sync.dma_start(out=srow[0:1, :, lo:hi],
                              in_=sph[:, 0 : n // 16])
            bch = smallp.tile([64, 2, QC], F16, name="bch", tag="bch")
            half = n // 2
            nc.gpsimd.partition_broadcast(
                out_ap=bch[:, :, lo : lo + half],
                in_ap=srow[0:1, :, lo : lo + half],
            )
            nc.gpsimd.partition_broadcast(
                out_ap=bch[:, :, lo + half : hi],
                in_ap=srow[0:1, :, lo + half : hi],
            )

            def div_fin(qc=qc, onum=onum, bch=bch, osb=osb, lo=lo, hi=hi):
                nc.vector.tensor_mul(
                    out=osb[0:64, qc * QC + lo : qc * QC + hi],
                    in0=onum[0:64, 0, lo:hi],
                    in1=bch[:, 0, lo:hi],
                )
                nc.vector.tensor_mul(
                    out=osb[64:128, qc * QC + lo : qc * QC + hi],
                    in0=onum[0:64, 1, lo:hi],
                    in1=bch[:, 1, lo:hi],
                )

            if defer:
                lq.append(("df", div_fin))
            else:
                div_fin()

        def tail_recip(st, o_ps, lo, hi):
            """Tail divide, stage 1 for columns [lo:hi): lift the final
            ones-row off PSUM (plain shifted DVE copy), approximate
            reciprocal at partition 0, partition-broadcast.  Emitted as
            soon as the columns' last attn@V is in flight."""
            rd = smallp.tile([1, 2, QC], F32, name="rd", tag="rd")
            nc.vector.tensor_copy(
                out=rd[0:1, :, lo:hi], in_=o_ps[64:65, :, lo:hi]
            )
            nc.vector.reciprocal_approx_fast(
                out=rd[0:1, :, lo:hi], in_=rd[0:1, :, lo:hi]
            )
            bch = smallp.tile([64, 2, QC], F32, name="bch", tag="bch")
            nc.gpsimd.partition_broadcast(
                out_ap=bch[:, :, lo:hi], in_ap=rd[0:1, :, lo:hi]
            )
            return bch

        def tail_muls(st, o_ps, bch, qc, lo, hi):
            """Tail divide, stage 2: the muls read the numerator straight
            from PSUM and write both head halves of osb."""
            osb = st["osb"]
            nc.vector.tensor_mul(
                out=osb[0:64, qc * QC + lo : qc * QC + hi],
                in0=o_ps[0:64, 0, lo:hi],
                in1=bch[:, 0, lo:hi],
            )
            nc.vector.tensor_mul(
                out=osb[64:128, qc * QC + lo : qc * QC + hi],
                in0=o_ps[0:64, 1, lo:hi],
                in1=bch[:, 1, lo:hi],
            )

        for b in range(B):
            cur_b[0] = b
            if b == 0:
                state[0] = alloc_batch(0)
                us, _ = stage_a_units(state[0], 0, prologue=True)
                for i, u in enumerate(us):
                    u(evac="scalar" if i % 2 else "vector")
                    if i == 0:
                        nc.sync.dma_start(out=tri_sb[:, :], in_=tri[:, :])
                    elif i == 2:
                        # w1 right after the first qk unit is emitted: its
                        # transfer hides under that unit's matmuls
                        nc.scalar.dma_start(
                            out=w_sb[:, 1, :, :], in_=wqkv[1, :, :, :]
                        )
                    elif i == 4:
                        nc.scalar.dma_start(
                            out=w_sb[:, 2, :, :], in_=wqkv[2, :, :, :]
                        )
                        nc.sync.dma_start(out=wo_sb[:, :], in_=wo[:, :])
            if b + 1 < B:
                state[b + 1] = alloc_batch(b + 1)
                us, defer = stage_a_units(
                    state[b + 1], b + 1, defer_tail=(b + 1 == B - 1)
                )
                aq.extend(us)
                b3q.extend(defer)

            st = state[b]
            qt, ktt, vsb, osb = st["qt"], st["ktt"], st["vsb"], st["osb"]

            for qc in range(NQC):
                o_ps = psO.tile([65, 2, QC], F32, name="o_ps", tag="o")
                nkb = 4 * qc + 4
                last_qc = b == B - 1 and qc == NQC - 1

                def emit_av(kb, off, n, p_t, o_ps=o_ps, nkb=nkb):
                    for h in range(2):
                        nc.tensor.matmul(
                            o_ps[:, h, off:QC],
                            lhsT=vsb[:, kb, h, 0:65],
                            rhs=p_t[:, h, 0:n],
                            start=(kb == 0),
                            stop=(kb == nkb - 1),
                            skip_group_check=True,
                        )

                pending = []
                for kb in range(nkb):
                    off = max(0, (kb - 4 * qc) * 128)
                    n = QC - off
                    s_ps = psS.tile([128, 2, QC], F32, name="s_ps", tag="s")
                    for h in range(2):
                        nc.tensor.matmul(
                            s_ps[:, h, 0:n],
                            lhsT=ktt[
                                64 * h : 64 * h + 64,
                                kb * 128 : (kb + 1) * 128,
                            ],
                            rhs=qt[
                                64 * h : 64 * h + 64,
                                qc * QC + off : (qc + 1) * QC,
                            ],
                            start=True,
                            stop=True,
                        )
                    p_t = ptp.tile([128, 2, QC], F16, name="p_t", tag="pt")
                    nc.scalar.activation(
                        out=p_t[:, :, 0:n], in_=s_ps[:, :, 0:n], func=Exp
                    )
                    if kb >= 4 * qc:
                        nc.vector.tensor_mul(
                            out=p_t[:, :, 0:128],
                            in0=p_t[:, :, 0:128],
                            in1=tri_sb[:, :]
                            .unsqueeze(1)
                            .broadcast_to([128, 2, 128]),
                        )
                    # filler keeps the PE queue full while the exp for
                    # this block is still in flight
                    pop_filler(kb)
                    pending.append((kb, off, n, p_t))
                    if len(pending) > 3:
                        emit_av(*pending.pop(0))

                if qc >= 1:
                    for half in range(2):
                        for u in oproj_unit(st, b, qc - 1, half):
                            lq.append(("op", u))

                if not last_qc:
                    for pv in pending:
                        emit_av(*pv)
                    divide_chain(st, o_ps, qc, 0, QC)
                else:
                    # tail: pipeline the divide + output projection in two
                    # 256-query halves so the drain chain is half as long.
                    # queries [0:256) are final after the aV for kb13.
                    # queries [0:256) of this qc are final after kb13's
                    # attn@V, so its reciprocal chain runs while the PE
                    # streams kb14/kb15 (the cheap row-copy briefly blocks
                    # kb14's accumulate; the rest is off the PE's path)
                    emit_av(*pending.pop(0))          # kb13
                    bch0 = tail_recip(st, o_ps, 0, QC // 2)
                    for pv in pending:                # kb14, kb15
                        emit_av(*pv)
                    tail_muls(st, o_ps, bch0, qc, 0, QC // 2)
                    bch1 = tail_recip(st, o_ps, QC // 2, QC)
                    for u in oproj_unit(st, b, qc, 0, split_q=True):
                        u()
                    tail_muls(st, o_ps, bch1, qc, QC // 2, QC)
                    for u in oproj_unit(st, b, qc, 1, split_q=True):
                        u()

            # batch end: emit reserved projection units (they keep the PE
            # busy while the last divide chain completes) and carry the
            # remaining lq work into the next batch's slots.  Their PSUM
            # evacuations alternate scalar/vector so the banks recycle
            # even while the divide chain occupies the DVE queue.
            ei = 0
            while aq:
                aq.popleft()(evac="scalar" if ei % 2 else "vector")
                ei += 1
            last = b == B - 1
            if not last:
                for half in range(2):
                    for u in oproj_unit(st, b, NQC - 1, half):
                        lq.append(("op", u))
            else:
                while lq:
                    lq.popleft()[1]()
            if b - 1 in state:
                del state[b - 1]


def _build(use_bias=False):
    nc = bacc.Bacc("TRN2", target_bir_lowering=False)
    xt = nc.dram_tensor("xt", [128, R // RC, KT, RC], F16, kind="ExternalInput")
    wqkv = nc.dram_tensor("wqkv", [3, 128, KT, HD], F16, kind="ExternalInput")
    bqkv = nc.dram_tensor("bqkv", [3, HD], F32, kind="ExternalInput")
    wo = nc.dram_tensor("wo", [HD, C], F16, kind="ExternalInput")
    tri = nc.dram_tensor("tri", [128, 128], F16, kind="ExternalInput")
    y = nc.dram_tensor("y", [R, C], F16, kind="ExternalOutput")
    with tile.TileContext(nc) as tc:
        _emit(nc, tc, xt, wqkv, bqkv, wo, tri, y, use_bias)
    nc.finalize()
    return nc


def kernel(hidden_states, w_qkv, b_qkv, w_o, b_o):
    global LAST_RESULT, _CACHED_NC
    X = np.ascontiguousarray(np.asarray(hidden_states, dtype=np.float32)).reshape(
        R, C
    )
    w_qkv = np.asarray(w_qkv, dtype=np.float32)
    b_qkv = np.asarray(b_qkv, dtype=np.float32)
    w_o = np.asarray(w_o, dtype=np.float32)
    b_o = np.asarray(b_o, dtype=np.float32)

    # [ki, rc, ko, col] layout: each partition's per-chunk read is one
    # contiguous 8 KB run, so the x_t DMAs stream at full bandwidth
    Xt = X.T.astype(np.float16).reshape(KT, 128, R // RC, RC)
    Xt = np.ascontiguousarray(Xt.transpose(1, 2, 0, 3))
    scale = float(DH) ** -0.5
    tri_m = np.triu(np.ones((128, 128), dtype=np.float32)).astype(np.float16)

    in_maps = []
    for c in range(NCORES):
        heads = [HPC * c + i for i in range(HPC)]
        wcols, bcols = [], []
        for sec in range(3):  # q, k, v
            sc = scale if sec == 0 else 1.0
            for h in heads:
                lo = sec * C + h * DH
                wcols.append(w_qkv[:, lo : lo + DH] * sc)
                bcols.append(b_qkv[lo : lo + DH] * sc)
        wqkv_c = (
            np.concatenate(wcols, axis=1)
            .astype(np.float16)
            .reshape(KT, 128, 3, HD)
        )
        wqkv_c = np.ascontiguousarray(wqkv_c.transpose(2, 1, 0, 3))
        bqkv_c = np.ascontiguousarray(np.concatenate(bcols).reshape(3, HD))
        wo_c = np.ascontiguousarray(
            np.concatenate([w_o[h * DH : (h + 1) * DH, :] for h in heads], axis=0)
        ).astype(np.float16)  # [HD, C]
        in_maps.append(
            {
                "xt": Xt,
                "wqkv": wqkv_c,
                "bqkv": bqkv_c,
                "wo": wo_c,
                "tri": tri_m,
            }
        )

    if _CACHED_NC is None:
        _CACHED_NC = _build(use_bias=bool(np.any(b_qkv)))
    res = run_bass_kernel_spmd(_CACHED_NC, in_maps, core_ids=list(range(NCORES)))
    LAST_RESULT = res

    out = res.results[0]["y"].astype(np.float64)
    for c in range(1, NCORES):
        out += res.results[c]["y"]
    out += b_o
    return out.astype(np.float32).reshape(B, T, C)
